# revision 15
# baseline (speedup 1.0000x reference)
"""BiLSTM-CRF NLL loss on 8 Trainium2 NeuronCores.

Sharding: core c in 0..7 -> direction = c//4 (0=fwd, 1=bwd on time-reversed
input), batch group = c%4 (8 sequences each). Each core: embedding gather ->
input transform -> LSTM recurrence (256 steps) -> partial emissions ->
pairwise AllReduce (fwd+bwd emission halves) -> exp-domain CRF forward
algorithm + gold-path score -> per-core loss partials. Host sums partials/32.

Self-contained: hardcodes all shapes; only needs numpy + concourse (+ml_dtypes).
"""
import numpy as np
import ml_dtypes

import concourse.bass as bass
import concourse.bacc as bacc
import concourse.tile as tile
from concourse import mybir
from concourse.bass_utils import run_bass_kernel_spmd

F32 = mybir.dt.float32
BF16 = mybir.dt.bfloat16
I32 = mybir.dt.int32
AF = mybir.ActivationFunctionType
ALU = mybir.AluOpType

B, S, E, H, T, V = 32, 256, 256, 512, 45, 50000
G4 = 4 * H          # 2048 gates
NB = 8              # sequences per core
N = S * NB          # 2048 rows, t-major: n = 8t+b
LN45 = float(np.log(45.0))

_cached = {}


def _phase_ab(nc, tc, gates_x, ones128, d):
    """Embedding gather + X transpose + input transform -> gates_x (bf16)."""
    with tc.tile_pool(name="ab", bufs=1) as ab:
        X = ab.tile([128, 16 * E], F32)
        XT = ab.tile([128, 2 * 2048], F32)
        wihT = ab.tile([128, 2 * G4], F32)
        bias1 = ab.tile([1, G4], F32)
        xidx = ab.tile([128, 16], I32)
        id128 = ab.tile([128, 128], F32)
        nc.sync.dma_start(out=wihT[:], in_=d["wihT"][:])
        nc.sync.dma_start(out=bias1[:], in_=d["bias1"][:])
        nc.sync.dma_start(out=xidx[:], in_=d["xidx"][:])
        nc.sync.dma_start(out=id128[:], in_=d["id128"][:])
        for j in range(16):
            nc.gpsimd.indirect_dma_start(
                out=X[:, 256 * j: 256 * j + 256],
                out_offset=None,
                in_=d["emb"][:],
                in_offset=bass.IndirectOffsetOnAxis(ap=xidx[:, j:j + 1], axis=0),
            )
        with tc.tile_pool(name="ps_tp", bufs=4, space="PSUM") as ps_tp:
            for j in range(16):
                for ec in range(2):
                    tp = ps_tp.tile([128, 128], F32, tag="tp")
                    nc.tensor.transpose(tp[:], X[:, 256 * j + 128 * ec: 256 * j + 128 * ec + 128], id128[:])
                    nc.vector.tensor_copy(XT[:, 2048 * ec + 128 * j: 2048 * ec + 128 * j + 128], tp[:])
        with tc.tile_pool(name="ps_gx", bufs=2, space="PSUM") as ps_gx:
            for j in range(16):
                gx = ps_gx.tile([128, G4], F32, tag="gx")
                for ng in range(4):
                    o = gx[:, 512 * ng: 512 * ng + 512]
                    nc.tensor.matmul(o, ones128[:], bias1[:, 512 * ng: 512 * ng + 512], start=True, stop=False)
                    for ec in range(2):
                        nc.tensor.matmul(
                            o,
                            XT[:, 2048 * ec + 128 * j: 2048 * ec + 128 * j + 128],
                            wihT[:, G4 * ec + 512 * ng: G4 * ec + 512 * ng + 512],
                            start=False, stop=(ec == 1))
                nc.vector.tensor_copy(gates_x[:, G4 * j: G4 * (j + 1)], gx[:])


def _phase_rec(nc, tc, gates_x, whhT, hsT, selb, id8, c0, c1):
    """LSTM recurrence, 256 steps; writes hsT (h transposed, col 8t+b).

    Gate order in packed weights: [i, f, o, g]. Per-step: 4 independent PSUM
    gate tiles (f first) -> per-gate activations -> bf16 cell update ->
    PE transpose of h -> hsT. bf16 datapath for 2x DVE and 2x PE streaming.
    """
    with tc.tile_pool(name="rec", bufs=3) as rp, \
         tc.tile_pool(name="ps_g", bufs=6, space="PSUM") as ps_g, \
         tc.tile_pool(name="ps_h", bufs=2, space="PSUM") as ps_h:
        cprev, cnext = c0, c1
        NGORD = (1, 0, 3, 2)   # f, i, g, o
        for t in range(S):
            m, u = t // 16, t % 16
            Gt = {}
            for ng in NGORD:
                Gn = ps_g.tile([NB, 512], F32, tag="G")
                Gt[ng] = Gn
                nc.tensor.matmul(
                    Gn[:], selb[:, 8 * u: 8 * u + 8],
                    gates_x[:, G4 * m + 512 * ng: G4 * m + 512 * ng + 512],
                    start=True, stop=(t == 0))
                if t > 0:
                    for kc in range(4):
                        nc.tensor.matmul(
                            Gn[:], hsT[:, G4 * kc + 8 * (t - 1): G4 * kc + 8 * (t - 1) + 8],
                            whhT[:, G4 * kc + 512 * ng: G4 * kc + 512 * ng + 512],
                            start=False, stop=(kc == 3))
            SG = rp.tile([NB, G4], BF16, tag="SG")
            nc.scalar.activation(SG[:, 512:1024], Gt[1][:], AF.Sigmoid)   # f
            nc.scalar.activation(SG[:, 0:512], Gt[0][:], AF.Sigmoid)     # i
            nc.scalar.activation(SG[:, 1536:2048], Gt[3][:], AF.Tanh)    # g
            nc.scalar.activation(SG[:, 1024:1536], Gt[2][:], AF.Sigmoid)  # o
            t1 = rp.tile([NB, H], BF16, tag="t1")
            t2 = rp.tile([NB, H], BF16, tag="t2")
            th = rp.tile([NB, H], BF16, tag="th")
            h = rp.tile([NB, H], BF16, tag="h")
            nc.vector.tensor_mul(t1[:], SG[:, 512:1024], cprev[:])
            nc.vector.tensor_mul(t2[:], SG[:, 0:512], SG[:, 1536:2048])
            nc.vector.tensor_add(cnext[:], t1[:], t2[:])
            nc.scalar.activation(th[:], cnext[:], AF.Tanh)
            nc.vector.tensor_mul(h[:], SG[:, 1024:1536], th[:])
            HT = ps_h.tile([128, 4 * NB], BF16, tag="HT")
            for kc in range(4):
                nc.tensor.transpose(HT[:, 8 * kc: 8 * kc + 8], h[:, 128 * kc: 128 * kc + 128], id8[:])
            hsT_v = hsT.rearrange("p (c n) -> p c n", c=4)[:, :, 8 * t: 8 * t + 8]
            nc.vector.tensor_copy(hsT_v, HT.rearrange("p (c n) -> p c n", c=4))
            cprev, cnext = cnext, cprev


def _phase_em(nc, tc, dp, hsT, ones128, emT, d):
    """Emissions (bt,45) -> per-core keep/swap permutation -> AllReduce -> emT (45,N)."""
    with tc.tile_pool(name="em", bufs=1) as ep, \
         tc.tile_pool(name="ps_em", bufs=2, space="PSUM") as ps_em:
        lin45 = ep.tile([128, 4 * T], BF16)
        linb1 = ep.tile([1, T], F32)
        keep = ep.tile([128, 128], F32)
        swap = ep.tile([128, 128], F32)
        id128b = ep.tile([128, 128], F32)
        emA = ep.tile([128, 16 * T], F32)
        emB = ep.tile([128, 16 * T], F32)
        emC = ep.tile([128, 16 * T], F32)
        nc.sync.dma_start(out=lin45[:], in_=d["lin45"][:])
        nc.sync.dma_start(out=linb1[:], in_=d["linb1"][:])
        nc.sync.dma_start(out=keep[:], in_=d["keepM"][:])
        nc.sync.dma_start(out=swap[:], in_=d["swapM"][:])
        nc.sync.dma_start(out=id128b[:], in_=d["id128"][:])
        for j in range(16):
            pe = ps_em.tile([128, T], F32, tag="pe")
            nc.tensor.matmul(pe[:], ones128[:], linb1[:], start=True, stop=False)
            for kc in range(4):
                nc.tensor.matmul(
                    pe[:], hsT[:, G4 * kc + 128 * j: G4 * kc + 128 * j + 128],
                    lin45[:, T * kc: T * kc + T], start=False, stop=(kc == 3))
            nc.vector.tensor_copy(emA[:, T * j: T * (j + 1)], pe[:])
        for j in range(16):
            pb = ps_em.tile([128, T], F32, tag="pb")
            nc.tensor.matmul(pb[:], keep[:], emA[:, T * j: T * (j + 1)], start=True, stop=False)
            nc.tensor.matmul(pb[:], swap[:], emA[:, T * (15 - j): T * (16 - j)], start=False, stop=True)
            nc.vector.tensor_copy(emB[:, T * j: T * (j + 1)], pb[:])

        bounceA = dp.tile([N, T], F32)
        bounceB = dp.tile([N, T], F32)
        # DRAM-side AP iterating (p, j, k) to match SBUF (partition, j, k)
        dstA = bass.AP(bounceA.tensor, 0, [[T, 128], [T * 128, 16], [1, T]])
        nc.sync.dma_start(out=dstA, in_=emB.rearrange("p (j k) -> p j k", j=16))
        nc.gpsimd.collective_compute(
            "AllReduce", ALU.add,
            replica_groups=[[0, 4], [1, 5], [2, 6], [3, 7]],
            ins=[bounceA.opt()], outs=[bounceB.opt()],
        )
        srcB = bass.AP(bounceB.tensor, 0, [[T, 128], [T * 128, 16], [1, T]])
        nc.sync.dma_start(out=emC.rearrange("p (j k) -> p j k", j=16), in_=srcB)
        for j in range(16):
            pt = ps_em.tile([T, 128], F32, tag="pt")
            nc.tensor.transpose(pt[:], emC[:, T * j: T * (j + 1)], id128b[:])
            nc.vector.tensor_copy(emT[:, 128 * j: 128 * (j + 1)], pt[:])


def _build(stop_after=None):
    lv = {"B": 1, "rec": 2, "em": 3, "crf": 4, None: 5}[stop_after]
    nc = bacc.Bacc("TRN2", target_bir_lowering=False, debug=False, num_devices=8)

    d = {}
    d["emb"] = nc.dram_tensor("emb", [V, E], F32, kind="ExternalInput")
    d["xidx"] = nc.dram_tensor("xidx", [128, 16], I32, kind="ExternalInput")
    d["wihT"] = nc.dram_tensor("wihT", [128, 2 * G4], F32, kind="ExternalInput")
    d["bias1"] = nc.dram_tensor("bias1", [1, G4], F32, kind="ExternalInput")
    d["whhT"] = nc.dram_tensor("whhT", [128, 4 * G4], BF16, kind="ExternalInput")
    d["sel"] = nc.dram_tensor("sel", [128, 128], BF16, kind="ExternalInput")
    d["id8"] = nc.dram_tensor("id8", [8, 8], BF16, kind="ExternalInput")
    d["id128"] = nc.dram_tensor("id128", [128, 128], F32, kind="ExternalInput")
    d["lin45"] = nc.dram_tensor("lin45", [128, 4 * T], BF16, kind="ExternalInput")
    d["linb1"] = nc.dram_tensor("linb1", [1, T], F32, kind="ExternalInput")
    d["keepM"] = nc.dram_tensor("keepM", [128, 128], F32, kind="ExternalInput")
    d["swapM"] = nc.dram_tensor("swapM", [128, 128], F32, kind="ExternalInput")
    d["trans"] = nc.dram_tensor("trans", [T, T], F32, kind="ExternalInput")
    d["stend"] = nc.dram_tensor("stend", [T, 2], F32, kind="ExternalInput")
    d["oh"] = nc.dram_tensor("oh", [T, N], F32, kind="ExternalInput")
    d["oh2"] = nc.dram_tensor("oh2", [T, N], F32, kind="ExternalInput")
    d_loss = nc.dram_tensor("loss", [1, NB], F32, kind="ExternalOutput")

    with tile.TileContext(nc) as tc:
        with tc.tile_pool(name="persist", bufs=1) as pp, \
             tc.tile_pool(name="dram", bufs=1, space="DRAM") as dp:
            gates_x = pp.tile([128, 16 * G4], BF16)
            whhT = pp.tile([128, 4 * G4], BF16)
            hsT = pp.tile([128, 4 * G4], BF16)
            selb = pp.tile([128, 128], BF16)
            id8 = pp.tile([8, 8], BF16)
            ones128 = pp.tile([1, 128], F32)
            c0 = pp.tile([8, H], BF16)
            c1 = pp.tile([8, H], BF16)

            nc.sync.dma_start(out=whhT[:], in_=d["whhT"][:])
            nc.sync.dma_start(out=selb[:], in_=d["sel"][:])
            nc.sync.dma_start(out=id8[:], in_=d["id8"][:])
            nc.vector.memset(ones128[:], 1.0)
            nc.vector.memset(c0[:], 0.0)

            _phase_ab(nc, tc, gates_x, ones128, d)
            if lv == 1:
                nc.sync.dma_start(out=d_loss[:], in_=ones128[:, 0:NB])

            if lv >= 2:
                _phase_rec(nc, tc, gates_x, whhT, hsT, selb, id8, c0, c1)
                if lv == 2:
                    nc.gpsimd.dma_start(out=d_loss[:], in_=hsT[0:1, 0:NB])

            emT = pp.tile([T, N], F32)
            if lv >= 3:
                _phase_em(nc, tc, dp, hsT, ones128, emT, d)
                if lv == 3:
                    nc.sync.dma_start(out=d_loss[:], in_=emT[0:1, 0:NB])

            if lv >= 4:
                with tc.tile_pool(name="crf", bufs=1) as cp, \
                     tc.tile_pool(name="ps_q", bufs=2, space="PSUM") as ps_q, \
                     tc.tile_pool(name="qs", bufs=2) as qp:
                    exp_em = cp.tile([T, N], F32)
                    trans_sb = cp.tile([T, T], F32)
                    stend = cp.tile([T, 2], F32)
                    Ep = cp.tile([T, T], F32)
                    estart = cp.tile([T, 1], F32)
                    eend = cp.tile([T, 1], F32)
                    ones45 = cp.tile([T, 1], F32)
                    nln45 = cp.tile([T, 1], F32)
                    nc.vector.memset(nln45[:], -LN45)
                    oh = cp.tile([T, N], F32)
                    oh2 = cp.tile([T, N], F32)
                    nc.sync.dma_start(out=oh2[:], in_=d["oh2"][:])
                    nc.sync.dma_start(out=trans_sb[:], in_=d["trans"][:])
                    nc.sync.dma_start(out=stend[:], in_=d["stend"][:])
                    nc.sync.dma_start(out=oh[:], in_=d["oh"][:])
                    nc.vector.memset(ones45[:], 1.0)
                    nc.scalar.activation(exp_em[:], emT[:], AF.Exp)
                    nc.scalar.activation(Ep[:], trans_sb[:], AF.Exp, bias=nln45[:])
                    nc.scalar.activation(estart[:], stend[:, 0:1], AF.Exp)
                    nc.scalar.activation(eend[:], stend[:, 1:2], AF.Exp)

                    q = qp.tile([T, NB], F32, tag="q")
                    nc.vector.tensor_scalar_mul(q[:], exp_em[:, 0:NB], estart[:])
                    for t in range(1, S):
                        sT = ps_q.tile([T, NB], F32, tag="sT")
                        nc.tensor.matmul(sT[:], Ep[:], q[:], start=True, stop=True)
                        qn = qp.tile([T, NB], F32, tag="q")
                        nc.vector.tensor_mul(qn[:], sT[:], exp_em[:, NB * t: NB * (t + 1)])
                        q = qn
                    if lv == 4:
                        nc.sync.dma_start(out=d_loss[:], in_=q[0:1, :])

                    if lv >= 5:
                        w = cp.tile([T, NB], F32)
                        nc.vector.tensor_scalar_mul(w[:], q[:], eend[:])
                        logZ = cp.tile([1, NB], F32)
                        em_sc = cp.tile([1, NB], F32)
                        tr_sc = cp.tile([1, NB], F32)
                        sten_s = cp.tile([1, NB], F32)
                        with tc.tile_pool(name="ps_f1", bufs=1, space="PSUM") as ps_f1:
                            sumw = ps_f1.tile([1, NB], F32, tag="f1")
                            nc.tensor.matmul(sumw[:], ones45[:], w[:], start=True, stop=True)
                            nc.scalar.activation(logZ[:], sumw[:], AF.Ln)
                        with tc.tile_pool(name="ps_s1", bufs=1, space="PSUM") as ps_s1:
                            S1 = cp.tile([T, N], F32)
                            nc.vector.tensor_mul(S1[:], emT[:], oh[:])
                            s1p = ps_s1.tile([1, N], F32, tag="fbig")
                            for ck in range(4):
                                nc.tensor.matmul(s1p[:, 512 * ck: 512 * ck + 512], ones45[:],
                                                 S1[:, 512 * ck: 512 * ck + 512], start=True, stop=True)
                            nc.vector.tensor_reduce(
                                em_sc[:], s1p.rearrange("p (t b) -> p b t", b=NB),
                                axis=mybir.AxisListType.X, op=ALU.add)
                        with tc.tile_pool(name="ps_R", bufs=1, space="PSUM") as ps_R:
                            Rp_ = ps_R.tile([T, N], F32, tag="fR")
                            for ck in range(4):
                                nc.tensor.matmul(Rp_[:, 512 * ck: 512 * ck + 512], trans_sb[:],
                                                 oh[:, 512 * ck: 512 * ck + 512], start=True, stop=True)
                            S2 = cp.tile([T, N], F32)
                            nc.vector.tensor_mul(S2[:], Rp_[:], oh2[:])
                        with tc.tile_pool(name="ps_s2", bufs=1, space="PSUM") as ps_s2:
                            s2p = ps_s2.tile([1, N], F32, tag="fbig")
                            for ck in range(4):
                                nc.tensor.matmul(s2p[:, 512 * ck: 512 * ck + 512], ones45[:],
                                                 S2[:, 512 * ck: 512 * ck + 512], start=True, stop=True)
                            nc.vector.tensor_reduce(
                                tr_sc[:], s2p.rearrange("p (t b) -> p b t", b=NB),
                                axis=mybir.AxisListType.X, op=ALU.add)
                        with tc.tile_pool(name="ps_f2", bufs=1, space="PSUM") as ps_f2:
                            stp = cp.tile([T, NB], F32)
                            enp = cp.tile([T, NB], F32)
                            nc.vector.tensor_scalar_mul(stp[:], oh[:, 0:NB], stend[:, 0:1])
                            nc.vector.tensor_scalar_mul(enp[:], oh[:, NB * (S - 1):N], stend[:, 1:2])
                            sten = ps_f2.tile([1, NB], F32, tag="f2")
                            nc.tensor.matmul(sten[:], ones45[:], stp[:], start=True, stop=False)
                            nc.tensor.matmul(sten[:], ones45[:], enp[:], start=False, stop=True)
                            nc.vector.tensor_copy(sten_s[:], sten[:])

                        sc1 = cp.tile([1, NB], F32)
                        sc2 = cp.tile([1, NB], F32)
                        lossa = cp.tile([1, NB], F32)
                        lossb = cp.tile([1, NB], F32)
                        nc.vector.tensor_add(sc1[:], em_sc[:], tr_sc[:])
                        nc.vector.tensor_add(sc2[:], sc1[:], sten_s[:])
                        nc.vector.tensor_tensor(out=lossa[:], in0=logZ[:], in1=sc2[:], op=ALU.subtract)
                        nc.scalar.activation(lossb[:], lossa[:], AF.Copy, bias=(S - 1) * LN45)
                        nc.sync.dma_start(out=d_loss[:], in_=lossb[:])

    nc.finalize()
    return nc


def _perm_gates(w):
    i, f, g, o = np.split(w, 4, axis=0)
    return np.concatenate([i, f, o, g], axis=0)


def _pack_k(wT, kchunks):
    # [K, M] -> [128, kchunks*M] with k-chunk kc at col block kc
    K, M = wT.shape
    assert K == 128 * kchunks
    return np.ascontiguousarray(wT.reshape(kchunks, 128, M).transpose(1, 0, 2).reshape(128, kchunks * M))


def prepare_in_maps(**inputs):
    x = np.asarray(inputs["x"]).astype(np.int32)          # [32, 256]
    tags = np.asarray(inputs["tags"]).astype(np.int32)
    emb = np.asarray(inputs["emb"], dtype=np.float32)
    lin_w = np.asarray(inputs["lin_w"], dtype=np.float32)
    lin_b = np.asarray(inputs["lin_b"], dtype=np.float32)
    start_t = np.asarray(inputs["start_t"], dtype=np.float32)
    end_t = np.asarray(inputs["end_t"], dtype=np.float32)
    trans = np.asarray(inputs["trans"], dtype=np.float32)

    wih = {0: _perm_gates(np.asarray(inputs["w_ih_f"], np.float32)),
           1: _perm_gates(np.asarray(inputs["w_ih_b"], np.float32))}
    whh = {0: _perm_gates(np.asarray(inputs["w_hh_f"], np.float32)),
           1: _perm_gates(np.asarray(inputs["w_hh_b"], np.float32))}
    bb = {0: _perm_gates(np.asarray(inputs["b_f"], np.float32)),
          1: _perm_gates(np.asarray(inputs["b_b"], np.float32))}

    id128 = np.eye(128, dtype=np.float32)
    id8 = np.eye(8, dtype=np.float32)
    u = np.arange(128)
    rev = 8 * (15 - (u // 8)) + (u % 8)
    revM = np.zeros((128, 128), np.float32)
    revM[u, rev[u]] = 1.0
    sel = np.eye(128, dtype=np.float32)   # col 8u+b selects row 8u+b

    in_maps = []
    for core in range(8):
        dirn, grp = core // 4, core % 4
        xs = x[8 * grp: 8 * grp + 8]
        if dirn == 1:
            xs = xs[:, ::-1]
        x_flat = xs.T.reshape(-1)                         # n = 8t+b
        xidx = np.ascontiguousarray(x_flat.reshape(16, 128).T).astype(np.int32)

        tg = tags[8 * grp: 8 * grp + 8]                   # [8, 256]
        oh = np.zeros((T, N), np.float32)
        oh[tg.T.reshape(-1), np.arange(N)] = 1.0
        oh2c = np.zeros((T, N), np.float32)
        oh2c[:, 0:N - NB] = oh[:, NB:N]

        lin_half = lin_w[:, 512 * dirn: 512 * (dirn + 1)]  # [45, 512]
        in_maps.append({
            "emb": emb,
            "xidx": xidx,
            "wihT": _pack_k(wih[dirn].T.copy(), 2),
            "bias1": bb[dirn].reshape(1, G4),
            "whhT": _pack_k(whh[dirn].T.copy(), 4).astype(ml_dtypes.bfloat16),
            "sel": sel.astype(ml_dtypes.bfloat16),
            "id8": id8.astype(ml_dtypes.bfloat16),
            "id128": id128,
            "lin45": _pack_k(np.ascontiguousarray(lin_half.T), 4).astype(ml_dtypes.bfloat16),
            "linb1": (lin_b if dirn == 0 else np.zeros_like(lin_b)).reshape(1, T),
            "keepM": id128 if dirn == 0 else np.zeros_like(id128),
            "swapM": np.zeros_like(revM) if dirn == 0 else revM,
            "trans": trans,
            "stend": np.stack([start_t, end_t], axis=1),
            "oh": oh if dirn == 0 else np.zeros_like(oh),
            "oh2": oh2c if dirn == 0 else np.zeros_like(oh),
        })

    return in_maps


def get_nc():
    if "nc" not in _cached:
        _cached["nc"] = _build()
    return _cached["nc"]


def kernel(**inputs):
    in_maps = prepare_in_maps(**inputs)
    res = run_bass_kernel_spmd(get_nc(), in_maps, core_ids=list(range(8)))
    total = np.float64(0.0)
    for core in range(4):
        total += np.float64(res.results[core]["loss"]).sum()
    return np.float32(total / 32.0)


# revision 23
# speedup vs baseline: 39.1793x; 39.1793x over previous
"""BiLSTM-CRF NLL loss on 8 Trainium2 NeuronCores.

Sharding: core c in 0..7 -> direction = c//4 (0=fwd, 1=bwd on time-reversed
input), batch group = c%4 (8 sequences each). Each core: embedding gather ->
input transform -> LSTM recurrence (256 steps) -> partial emissions ->
pairwise AllReduce (fwd+bwd emission halves) -> exp-domain CRF forward
algorithm + gold-path score -> per-core loss partials. Host sums partials/32.

Self-contained: hardcodes all shapes; only needs numpy + concourse (+ml_dtypes).
"""
import numpy as np
import ml_dtypes

import concourse.bass as bass
import concourse.bacc as bacc
import concourse.tile as tile
from concourse.tile_rust import add_dep_helper
from concourse import mybir
from concourse.bass_utils import run_bass_kernel_spmd

F32 = mybir.dt.float32
BF16 = mybir.dt.bfloat16
I32 = mybir.dt.int32
AF = mybir.ActivationFunctionType
ALU = mybir.AluOpType

B, S, E, H, T, V = 32, 256, 256, 512, 45, 50000
G4 = 4 * H          # 2048 gates
NB = 8              # sequences per core
N = S * NB          # 2048 rows, t-major: n = 8t+b
LN45 = float(np.log(45.0))

_cached = {}


def _phase_ab(nc, tc, gates_x, ones128, d):
    """Embedding gather + X transpose + input transform -> gates_x (bf16)."""
    with tc.tile_pool(name="ab", bufs=1) as ab:
        X = ab.tile([128, 16 * E], F32)
        XT = ab.tile([128, 2 * 2048], F32)
        wihT = ab.tile([128, 2 * G4], F32)
        bias1 = ab.tile([1, G4], F32)
        xidx = ab.tile([128, 16], I32)
        id128 = ab.tile([128, 128], F32)
        nc.sync.dma_start(out=wihT[:], in_=d["wihT"][:])
        nc.sync.dma_start(out=bias1[:], in_=d["bias1"][:])
        nc.sync.dma_start(out=xidx[:], in_=d["xidx"][:])
        nc.sync.dma_start(out=id128[:], in_=d["id128"][:])
        for j in range(16):
            nc.gpsimd.indirect_dma_start(
                out=X[:, 256 * j: 256 * j + 256],
                out_offset=None,
                in_=d["emb"][:],
                in_offset=bass.IndirectOffsetOnAxis(ap=xidx[:, j:j + 1], axis=0),
            )
        with tc.tile_pool(name="ps_tp", bufs=4, space="PSUM") as ps_tp:
            for j in range(16):
                for ec in range(2):
                    tp = ps_tp.tile([128, 128], F32, tag="tp")
                    nc.tensor.transpose(tp[:], X[:, 256 * j + 128 * ec: 256 * j + 128 * ec + 128], id128[:])
                    nc.vector.tensor_copy(XT[:, 2048 * ec + 128 * j: 2048 * ec + 128 * j + 128], tp[:])
        with tc.tile_pool(name="ps_gx", bufs=2, space="PSUM") as ps_gx:
            for j in range(16):
                gx = ps_gx.tile([128, G4], F32, tag="gx")
                for ng in range(4):
                    o = gx[:, 512 * ng: 512 * ng + 512]
                    nc.tensor.matmul(o, ones128[:], bias1[:, 512 * ng: 512 * ng + 512], start=True, stop=False)
                    for ec in range(2):
                        nc.tensor.matmul(
                            o,
                            XT[:, 2048 * ec + 128 * j: 2048 * ec + 128 * j + 128],
                            wihT[:, G4 * ec + 512 * ng: G4 * ec + 512 * ng + 512],
                            start=False, stop=(ec == 1))
                nc.vector.tensor_copy(gates_x[:, G4 * j: G4 * (j + 1)], gx[:])


def _phase_rec(nc, tc, gates_x, whhT, hsT, selb, id8, c0, c1):
    """LSTM recurrence, 256 steps; writes hsT (h transposed, col 8t+b).

    Gate order in packed weights: [i, f, o, g]. Per-step: 4 independent PSUM
    gate tiles (f first) -> per-gate activations -> bf16 cell update ->
    PE transpose of h -> hsT. bf16 datapath for 2x DVE and 2x PE streaming.
    """
    with tc.tile_pool(name="rec", bufs=3) as rp, \
         tc.tile_pool(name="ps_g", bufs=8, space="PSUM") as ps_g:
        cprev, cnext = c0, c1
        NGORD = (1, 0, 3, 2)   # f, i, g, o

        def emit_sel(t):
            # inject x-gates for step t (start=True opens each PSUM bank);
            # hoisted to fill the PE bubble while step t-1's tail drains
            m, u = t // 16, t % 16
            Gt = {}
            sel_ins = []
            for ng in NGORD:
                Gn = ps_g.tile([NB, 512], F32, tag="G")
                Gt[ng] = Gn
                mm = nc.tensor.matmul(
                    Gn[:], selb[:, 8 * u: 8 * u + 8],
                    gates_x[:, G4 * m + 512 * ng: G4 * m + 512 * ng + 512],
                    start=True, stop=(t == 0))
                sel_ins.append(mm.ins)
            return Gt, sel_ins

        Gnext, _ = emit_sel(0)
        for t in range(S):
            Gt = Gnext
            if t > 0:
                for ng in NGORD:
                    for kc in range(4):
                        nc.tensor.matmul(
                            Gt[ng][:], hsT[:, G4 * kc + 8 * (t - 1): G4 * kc + 8 * (t - 1) + 8],
                            whhT[:, G4 * kc + 512 * ng: G4 * kc + 512 * ng + 512],
                            start=False, stop=(kc == 3))
            SG = rp.tile([NB, G4], BF16, tag="SG")
            nc.scalar.activation(SG[:, 512:1024], Gt[1][:], AF.Sigmoid)   # f
            nc.scalar.activation(SG[:, 0:512], Gt[0][:], AF.Sigmoid)     # i
            nc.scalar.activation(SG[:, 1536:2048], Gt[3][:], AF.Tanh)    # g
            sel_next = []
            if t + 1 < S:
                Gnext, sel_next = emit_sel(t + 1)
            t1 = rp.tile([NB, H], BF16, tag="t1")
            t2 = rp.tile([NB, H], BF16, tag="t2")
            th = rp.tile([NB, H], BF16, tag="th")
            h = rp.tile([NB, H], BF16, tag="h")
            nc.vector.tensor_mul(t1[:], SG[:, 512:1024], cprev[:])
            nc.vector.tensor_mul(t2[:], SG[:, 0:512], SG[:, 1536:2048])
            nc.vector.tensor_add(cnext[:], t1[:], t2[:])
            nc.scalar.activation(th[:], cnext[:], AF.Tanh)
            # tail: chunked o-sigmoid / h / transpose / hsT copy so next-step
            # matmuls restart as soon as the first hsT half lands
            HT = ps_g.tile([128, 4 * NB], BF16, tag="G")
            hsT_v = hsT.rearrange("p (c n) -> p c n", c=4)
            HT_v = HT.rearrange("p (c n) -> p c n", c=4)
            for cc in range(2):
                sl = slice(256 * cc, 256 * cc + 256)
                nc.scalar.activation(SG[:, 1024 + 256 * cc: 1024 + 256 * cc + 256],
                                     Gt[2][:, sl], AF.Sigmoid)             # o half
                nc.vector.tensor_mul(h[:, sl], SG[:, 1024 + 256 * cc: 1024 + 256 * cc + 256], th[:, sl])
                for kc in (2 * cc, 2 * cc + 1):
                    tr = nc.tensor.transpose(HT[:, 8 * kc: 8 * kc + 8], h[:, 128 * kc: 128 * kc + 128], id8[:])
                    for si in sel_next:
                        add_dep_helper(tr.ins, si, sync=False,
                                       reason="order next-step x-gate MMs into the tail bubble")
                nc.vector.tensor_copy(hsT_v[:, 2 * cc: 2 * cc + 2, 8 * t: 8 * t + 8],
                                      HT_v[:, 2 * cc: 2 * cc + 2, :])
            cprev, cnext = cnext, cprev


def _phase_em(nc, tc, dp, hsT, ones128, emT, d):
    """Emissions (bt,45) -> per-core keep/swap permutation -> AllReduce -> emT (45,N)."""
    with tc.tile_pool(name="em", bufs=1) as ep, \
         tc.tile_pool(name="ps_em", bufs=2, space="PSUM") as ps_em:
        lin45 = ep.tile([128, 4 * T], BF16)
        linb1 = ep.tile([1, T], F32)
        keep = ep.tile([128, 128], F32)
        swap = ep.tile([128, 128], F32)
        id128b = ep.tile([128, 128], F32)
        emA = ep.tile([128, 16 * T], F32)
        emB = ep.tile([128, 16 * T], F32)
        emC = ep.tile([128, 16 * T], F32)
        nc.sync.dma_start(out=lin45[:], in_=d["lin45"][:])
        nc.sync.dma_start(out=linb1[:], in_=d["linb1"][:])
        nc.sync.dma_start(out=keep[:], in_=d["keepM"][:])
        nc.sync.dma_start(out=swap[:], in_=d["swapM"][:])
        nc.sync.dma_start(out=id128b[:], in_=d["id128"][:])
        for j in range(16):
            pe = ps_em.tile([128, T], F32, tag="pe")
            nc.tensor.matmul(pe[:], ones128[:], linb1[:], start=True, stop=False)
            for kc in range(4):
                nc.tensor.matmul(
                    pe[:], hsT[:, G4 * kc + 128 * j: G4 * kc + 128 * j + 128],
                    lin45[:, T * kc: T * kc + T], start=False, stop=(kc == 3))
            nc.vector.tensor_copy(emA[:, T * j: T * (j + 1)], pe[:])
        for j in range(16):
            pb = ps_em.tile([128, T], F32, tag="pb")
            nc.tensor.matmul(pb[:], keep[:], emA[:, T * j: T * (j + 1)], start=True, stop=False)
            nc.tensor.matmul(pb[:], swap[:], emA[:, T * (15 - j): T * (16 - j)], start=False, stop=True)
            nc.vector.tensor_copy(emB[:, T * j: T * (j + 1)], pb[:])

        bounceA = dp.tile([N, T], F32)
        bounceB = dp.tile([N, T], F32)
        # DRAM-side AP iterating (p, j, k) to match SBUF (partition, j, k)
        dstA = bass.AP(bounceA.tensor, 0, [[T, 128], [T * 128, 16], [1, T]])
        nc.sync.dma_start(out=dstA, in_=emB.rearrange("p (j k) -> p j k", j=16))
        nc.gpsimd.collective_compute(
            "AllReduce", ALU.add,
            replica_groups=[[0, 4], [1, 5], [2, 6], [3, 7]],
            ins=[bounceA.opt()], outs=[bounceB.opt()],
        )
        srcB = bass.AP(bounceB.tensor, 0, [[T, 128], [T * 128, 16], [1, T]])
        nc.sync.dma_start(out=emC.rearrange("p (j k) -> p j k", j=16), in_=srcB)
        for j in range(16):
            pt = ps_em.tile([T, 128], F32, tag="pt")
            nc.tensor.transpose(pt[:], emC[:, T * j: T * (j + 1)], id128b[:])
            nc.vector.tensor_copy(emT[:, 128 * j: 128 * (j + 1)], pt[:])


def _build(stop_after=None):
    lv = {"B": 1, "rec": 2, "em": 3, "crf": 4, None: 5}[stop_after]
    nc = bacc.Bacc("TRN2", target_bir_lowering=False, debug=False, num_devices=8)

    d = {}
    d["emb"] = nc.dram_tensor("emb", [V, E], F32, kind="ExternalInput")
    d["xidx"] = nc.dram_tensor("xidx", [128, 16], I32, kind="ExternalInput")
    d["wihT"] = nc.dram_tensor("wihT", [128, 2 * G4], F32, kind="ExternalInput")
    d["bias1"] = nc.dram_tensor("bias1", [1, G4], F32, kind="ExternalInput")
    d["whhT"] = nc.dram_tensor("whhT", [128, 4 * G4], BF16, kind="ExternalInput")
    d["sel"] = nc.dram_tensor("sel", [128, 128], BF16, kind="ExternalInput")
    d["id8"] = nc.dram_tensor("id8", [8, 8], BF16, kind="ExternalInput")
    d["id128"] = nc.dram_tensor("id128", [128, 128], F32, kind="ExternalInput")
    d["lin45"] = nc.dram_tensor("lin45", [128, 4 * T], BF16, kind="ExternalInput")
    d["linb1"] = nc.dram_tensor("linb1", [1, T], F32, kind="ExternalInput")
    d["keepM"] = nc.dram_tensor("keepM", [128, 128], F32, kind="ExternalInput")
    d["swapM"] = nc.dram_tensor("swapM", [128, 128], F32, kind="ExternalInput")
    d["trans"] = nc.dram_tensor("trans", [T, T], F32, kind="ExternalInput")
    d["stend"] = nc.dram_tensor("stend", [T, 2], F32, kind="ExternalInput")
    d["oh"] = nc.dram_tensor("oh", [T, N], F32, kind="ExternalInput")
    d["oh2"] = nc.dram_tensor("oh2", [T, N], F32, kind="ExternalInput")
    d_loss = nc.dram_tensor("loss", [1, NB], F32, kind="ExternalOutput")

    with tile.TileContext(nc) as tc:
        with tc.tile_pool(name="persist", bufs=1) as pp, \
             tc.tile_pool(name="dram", bufs=1, space="DRAM") as dp:
            gates_x = pp.tile([128, 16 * G4], BF16)
            whhT = pp.tile([128, 4 * G4], BF16)
            hsT = pp.tile([128, 4 * G4], BF16)
            selb = pp.tile([128, 128], BF16)
            id8 = pp.tile([8, 8], BF16)
            ones128 = pp.tile([1, 128], F32)
            c0 = pp.tile([8, H], BF16)
            c1 = pp.tile([8, H], BF16)

            nc.sync.dma_start(out=whhT[:], in_=d["whhT"][:])
            nc.sync.dma_start(out=selb[:], in_=d["sel"][:])
            nc.sync.dma_start(out=id8[:], in_=d["id8"][:])
            nc.vector.memset(ones128[:], 1.0)
            nc.vector.memset(c0[:], 0.0)

            _phase_ab(nc, tc, gates_x, ones128, d)
            if lv == 1:
                nc.sync.dma_start(out=d_loss[:], in_=ones128[:, 0:NB])

            if lv >= 2:
                _phase_rec(nc, tc, gates_x, whhT, hsT, selb, id8, c0, c1)
                if lv == 2:
                    nc.gpsimd.dma_start(out=d_loss[:], in_=hsT[0:1, 0:NB])

            emT = pp.tile([T, N], F32)
            if lv >= 3:
                _phase_em(nc, tc, dp, hsT, ones128, emT, d)
                if lv == 3:
                    nc.sync.dma_start(out=d_loss[:], in_=emT[0:1, 0:NB])

            if lv >= 4:
                with tc.tile_pool(name="crf", bufs=1) as cp, \
                     tc.tile_pool(name="ps_q", bufs=2, space="PSUM") as ps_q, \
                     tc.tile_pool(name="qs", bufs=2) as qp:
                    exp_em = cp.tile([T, N], F32)
                    trans_sb = cp.tile([T, T], F32)
                    stend = cp.tile([T, 2], F32)
                    Ep = cp.tile([T, T], F32)
                    estart = cp.tile([T, 1], F32)
                    eend = cp.tile([T, 1], F32)
                    ones45 = cp.tile([T, 1], F32)
                    nln45 = cp.tile([T, 1], F32)
                    nc.vector.memset(nln45[:], -LN45)
                    oh = cp.tile([T, N], F32)
                    oh2 = cp.tile([T, N], F32)
                    nc.sync.dma_start(out=oh2[:], in_=d["oh2"][:])
                    nc.sync.dma_start(out=trans_sb[:], in_=d["trans"][:])
                    nc.sync.dma_start(out=stend[:], in_=d["stend"][:])
                    nc.sync.dma_start(out=oh[:], in_=d["oh"][:])
                    nc.vector.memset(ones45[:], 1.0)
                    nc.scalar.activation(exp_em[:], emT[:], AF.Exp)
                    nc.scalar.activation(Ep[:], trans_sb[:], AF.Exp, bias=nln45[:])
                    nc.scalar.activation(estart[:], stend[:, 0:1], AF.Exp)
                    nc.scalar.activation(eend[:], stend[:, 1:2], AF.Exp)

                    q = qp.tile([T, NB], F32, tag="q")
                    nc.vector.tensor_scalar_mul(q[:], exp_em[:, 0:NB], estart[:])
                    for t in range(1, S):
                        sT = ps_q.tile([T, NB], F32, tag="sT")
                        nc.tensor.matmul(sT[:], Ep[:], q[:], start=True, stop=True)
                        qn = qp.tile([T, NB], F32, tag="q")
                        nc.vector.tensor_mul(qn[:], sT[:], exp_em[:, NB * t: NB * (t + 1)])
                        q = qn
                    if lv == 4:
                        nc.sync.dma_start(out=d_loss[:], in_=q[0:1, :])

                    if lv >= 5:
                        w = cp.tile([T, NB], F32)
                        nc.vector.tensor_scalar_mul(w[:], q[:], eend[:])
                        logZ = cp.tile([1, NB], F32)
                        em_sc = cp.tile([1, NB], F32)
                        tr_sc = cp.tile([1, NB], F32)
                        sten_s = cp.tile([1, NB], F32)
                        with tc.tile_pool(name="ps_f1", bufs=1, space="PSUM") as ps_f1:
                            sumw = ps_f1.tile([1, NB], F32, tag="f1")
                            nc.tensor.matmul(sumw[:], ones45[:], w[:], start=True, stop=True)
                            nc.scalar.activation(logZ[:], sumw[:], AF.Ln)
                        with tc.tile_pool(name="ps_s1", bufs=1, space="PSUM") as ps_s1:
                            S1 = cp.tile([T, N], F32)
                            nc.vector.tensor_mul(S1[:], emT[:], oh[:])
                            s1p = ps_s1.tile([1, N], F32, tag="fbig")
                            for ck in range(4):
                                nc.tensor.matmul(s1p[:, 512 * ck: 512 * ck + 512], ones45[:],
                                                 S1[:, 512 * ck: 512 * ck + 512], start=True, stop=True)
                            nc.vector.tensor_reduce(
                                em_sc[:], s1p.rearrange("p (t b) -> p b t", b=NB),
                                axis=mybir.AxisListType.X, op=ALU.add)
                        with tc.tile_pool(name="ps_R", bufs=1, space="PSUM") as ps_R:
                            Rp_ = ps_R.tile([T, N], F32, tag="fR")
                            for ck in range(4):
                                nc.tensor.matmul(Rp_[:, 512 * ck: 512 * ck + 512], trans_sb[:],
                                                 oh[:, 512 * ck: 512 * ck + 512], start=True, stop=True)
                            S2 = cp.tile([T, N], F32)
                            nc.vector.tensor_mul(S2[:], Rp_[:], oh2[:])
                        with tc.tile_pool(name="ps_s2", bufs=1, space="PSUM") as ps_s2:
                            s2p = ps_s2.tile([1, N], F32, tag="fbig")
                            for ck in range(4):
                                nc.tensor.matmul(s2p[:, 512 * ck: 512 * ck + 512], ones45[:],
                                                 S2[:, 512 * ck: 512 * ck + 512], start=True, stop=True)
                            nc.vector.tensor_reduce(
                                tr_sc[:], s2p.rearrange("p (t b) -> p b t", b=NB),
                                axis=mybir.AxisListType.X, op=ALU.add)
                        with tc.tile_pool(name="ps_f2", bufs=1, space="PSUM") as ps_f2:
                            stp = cp.tile([T, NB], F32)
                            enp = cp.tile([T, NB], F32)
                            nc.vector.tensor_scalar_mul(stp[:], oh[:, 0:NB], stend[:, 0:1])
                            nc.vector.tensor_scalar_mul(enp[:], oh[:, NB * (S - 1):N], stend[:, 1:2])
                            sten = ps_f2.tile([1, NB], F32, tag="f2")
                            nc.tensor.matmul(sten[:], ones45[:], stp[:], start=True, stop=False)
                            nc.tensor.matmul(sten[:], ones45[:], enp[:], start=False, stop=True)
                            nc.vector.tensor_copy(sten_s[:], sten[:])

                        sc1 = cp.tile([1, NB], F32)
                        sc2 = cp.tile([1, NB], F32)
                        lossa = cp.tile([1, NB], F32)
                        lossb = cp.tile([1, NB], F32)
                        nc.vector.tensor_add(sc1[:], em_sc[:], tr_sc[:])
                        nc.vector.tensor_add(sc2[:], sc1[:], sten_s[:])
                        nc.vector.tensor_tensor(out=lossa[:], in0=logZ[:], in1=sc2[:], op=ALU.subtract)
                        nc.scalar.activation(lossb[:], lossa[:], AF.Copy, bias=(S - 1) * LN45)
                        nc.sync.dma_start(out=d_loss[:], in_=lossb[:])

    nc.finalize()
    return nc


def _perm_gates(w):
    i, f, g, o = np.split(w, 4, axis=0)
    return np.concatenate([i, f, o, g], axis=0)


def _pack_k(wT, kchunks):
    # [K, M] -> [128, kchunks*M] with k-chunk kc at col block kc
    K, M = wT.shape
    assert K == 128 * kchunks
    return np.ascontiguousarray(wT.reshape(kchunks, 128, M).transpose(1, 0, 2).reshape(128, kchunks * M))


def prepare_in_maps(**inputs):
    x = np.asarray(inputs["x"]).astype(np.int32)          # [32, 256]
    tags = np.asarray(inputs["tags"]).astype(np.int32)
    emb = np.asarray(inputs["emb"], dtype=np.float32)
    lin_w = np.asarray(inputs["lin_w"], dtype=np.float32)
    lin_b = np.asarray(inputs["lin_b"], dtype=np.float32)
    start_t = np.asarray(inputs["start_t"], dtype=np.float32)
    end_t = np.asarray(inputs["end_t"], dtype=np.float32)
    trans = np.asarray(inputs["trans"], dtype=np.float32)

    wih = {0: _perm_gates(np.asarray(inputs["w_ih_f"], np.float32)),
           1: _perm_gates(np.asarray(inputs["w_ih_b"], np.float32))}
    whh = {0: _perm_gates(np.asarray(inputs["w_hh_f"], np.float32)),
           1: _perm_gates(np.asarray(inputs["w_hh_b"], np.float32))}
    bb = {0: _perm_gates(np.asarray(inputs["b_f"], np.float32)),
          1: _perm_gates(np.asarray(inputs["b_b"], np.float32))}

    id128 = np.eye(128, dtype=np.float32)
    id8 = np.eye(8, dtype=np.float32)
    u = np.arange(128)
    rev = 8 * (15 - (u // 8)) + (u % 8)
    revM = np.zeros((128, 128), np.float32)
    revM[u, rev[u]] = 1.0
    sel = np.eye(128, dtype=np.float32)   # col 8u+b selects row 8u+b

    in_maps = []
    for core in range(8):
        dirn, grp = core // 4, core % 4
        xs = x[8 * grp: 8 * grp + 8]
        if dirn == 1:
            xs = xs[:, ::-1]
        x_flat = xs.T.reshape(-1)                         # n = 8t+b
        xidx = np.ascontiguousarray(x_flat.reshape(16, 128).T).astype(np.int32)

        tg = tags[8 * grp: 8 * grp + 8]                   # [8, 256]
        oh = np.zeros((T, N), np.float32)
        oh[tg.T.reshape(-1), np.arange(N)] = 1.0
        oh2c = np.zeros((T, N), np.float32)
        oh2c[:, 0:N - NB] = oh[:, NB:N]

        lin_half = lin_w[:, 512 * dirn: 512 * (dirn + 1)]  # [45, 512]
        in_maps.append({
            "emb": emb,
            "xidx": xidx,
            "wihT": _pack_k(wih[dirn].T.copy(), 2),
            "bias1": bb[dirn].reshape(1, G4),
            "whhT": _pack_k(whh[dirn].T.copy(), 4).astype(ml_dtypes.bfloat16),
            "sel": sel.astype(ml_dtypes.bfloat16),
            "id8": id8.astype(ml_dtypes.bfloat16),
            "id128": id128,
            "lin45": _pack_k(np.ascontiguousarray(lin_half.T), 4).astype(ml_dtypes.bfloat16),
            "linb1": (lin_b if dirn == 0 else np.zeros_like(lin_b)).reshape(1, T),
            "keepM": id128 if dirn == 0 else np.zeros_like(id128),
            "swapM": np.zeros_like(revM) if dirn == 0 else revM,
            "trans": trans,
            "stend": np.stack([start_t, end_t], axis=1),
            "oh": oh if dirn == 0 else np.zeros_like(oh),
            "oh2": oh2c if dirn == 0 else np.zeros_like(oh),
        })

    return in_maps


def get_nc():
    if "nc" not in _cached:
        _cached["nc"] = _build()
    return _cached["nc"]


def kernel(**inputs):
    in_maps = prepare_in_maps(**inputs)
    res = run_bass_kernel_spmd(get_nc(), in_maps, core_ids=list(range(8)))
    total = np.float64(0.0)
    for core in range(4):
        total += np.float64(res.results[core]["loss"]).sum()
    return np.float32(total / 32.0)


# revision 24
# speedup vs baseline: 39.1908x; 1.0003x over previous
"""BiLSTM-CRF NLL loss on 8 Trainium2 NeuronCores.

Sharding: core c in 0..7 -> direction = c//4 (0=fwd, 1=bwd on time-reversed
input), batch group = c%4 (8 sequences each). Each core: embedding gather ->
input transform -> LSTM recurrence (256 steps) -> partial emissions ->
pairwise AllReduce (fwd+bwd emission halves) -> exp-domain CRF forward
algorithm + gold-path score -> per-core loss partials. Host sums partials/32.

Self-contained: hardcodes all shapes; only needs numpy + concourse (+ml_dtypes).
"""
import numpy as np
import ml_dtypes

import concourse.bass as bass
import concourse.bacc as bacc
import concourse.tile as tile
from concourse.tile_rust import add_dep_helper
from concourse import mybir
from concourse.bass_utils import run_bass_kernel_spmd

F32 = mybir.dt.float32
BF16 = mybir.dt.bfloat16
I32 = mybir.dt.int32
AF = mybir.ActivationFunctionType
ALU = mybir.AluOpType

B, S, E, H, T, V = 32, 256, 256, 512, 45, 50000
G4 = 4 * H          # 2048 gates
NB = 8              # sequences per core
N = S * NB          # 2048 rows, t-major: n = 8t+b
LN45 = float(np.log(45.0))

_cached = {}


def _phase_ab(nc, tc, gates_x, ones128, d):
    """Embedding gather + X transpose + input transform -> gates_x (bf16)."""
    with tc.tile_pool(name="ab", bufs=1) as ab:
        X = ab.tile([128, 16 * E], F32)
        XT = ab.tile([128, 2 * 2048], F32)
        wihT = ab.tile([128, 2 * G4], F32)
        bias1 = ab.tile([1, G4], F32)
        xidx = ab.tile([128, 16], I32)
        id128 = ab.tile([128, 128], F32)
        nc.sync.dma_start(out=wihT[:], in_=d["wihT"][:])
        nc.sync.dma_start(out=bias1[:], in_=d["bias1"][:])
        nc.sync.dma_start(out=xidx[:], in_=d["xidx"][:])
        nc.sync.dma_start(out=id128[:], in_=d["id128"][:])
        for j in range(16):
            nc.gpsimd.indirect_dma_start(
                out=X[:, 256 * j: 256 * j + 256],
                out_offset=None,
                in_=d["emb"][:],
                in_offset=bass.IndirectOffsetOnAxis(ap=xidx[:, j:j + 1], axis=0),
            )
        with tc.tile_pool(name="ps_tp", bufs=4, space="PSUM") as ps_tp:
            for j in range(16):
                for ec in range(2):
                    tp = ps_tp.tile([128, 128], F32, tag="tp")
                    nc.tensor.transpose(tp[:], X[:, 256 * j + 128 * ec: 256 * j + 128 * ec + 128], id128[:])
                    nc.vector.tensor_copy(XT[:, 2048 * ec + 128 * j: 2048 * ec + 128 * j + 128], tp[:])
        with tc.tile_pool(name="ps_gx", bufs=2, space="PSUM") as ps_gx:
            for j in range(16):
                gx = ps_gx.tile([128, G4], F32, tag="gx")
                for ng in range(4):
                    o = gx[:, 512 * ng: 512 * ng + 512]
                    nc.tensor.matmul(o, ones128[:], bias1[:, 512 * ng: 512 * ng + 512], start=True, stop=False)
                    for ec in range(2):
                        nc.tensor.matmul(
                            o,
                            XT[:, 2048 * ec + 128 * j: 2048 * ec + 128 * j + 128],
                            wihT[:, G4 * ec + 512 * ng: G4 * ec + 512 * ng + 512],
                            start=False, stop=(ec == 1))
                nc.vector.tensor_copy(gates_x[:, G4 * j: G4 * (j + 1)], gx[:])


def _phase_rec(nc, tc, gates_x, whhT, hsT, selb, id8, c0, c1):
    """LSTM recurrence, 256 steps; writes hsT (h transposed, col 8t+b).

    Gate order in packed weights: [i, f, o, g]. Per-step: 4 independent PSUM
    gate tiles (f first) -> per-gate activations -> bf16 cell update ->
    PE transpose of h -> hsT. bf16 datapath for 2x DVE and 2x PE streaming.
    """
    with tc.tile_pool(name="rec", bufs=3) as rp, \
         tc.tile_pool(name="ps_g", bufs=8, space="PSUM") as ps_g:
        cprev, cnext = c0, c1
        NGORD = (1, 0, 3, 2)   # f, i, g, o

        def emit_sel(t):
            # inject x-gates for step t (start=True opens each PSUM bank);
            # hoisted to fill the PE bubble while step t-1's tail drains
            m, u = t // 16, t % 16
            Gt = {}
            sel_ins = []
            for ng in NGORD:
                Gn = ps_g.tile([NB, 512], F32, tag="G")
                Gt[ng] = Gn
                mm = nc.tensor.matmul(
                    Gn[:], selb[:, 8 * u: 8 * u + 8],
                    gates_x[:, G4 * m + 512 * ng: G4 * m + 512 * ng + 512],
                    start=True, stop=(t == 0))
                sel_ins.append(mm.ins)
            return Gt, sel_ins

        Gnext, _ = emit_sel(0)
        for t in range(S):
            Gt = Gnext
            if t > 0:
                for ng in NGORD:
                    for kc in range(4):
                        nc.tensor.matmul(
                            Gt[ng][:], hsT[:, G4 * kc + 8 * (t - 1): G4 * kc + 8 * (t - 1) + 8],
                            whhT[:, G4 * kc + 512 * ng: G4 * kc + 512 * ng + 512],
                            start=False, stop=(kc == 3))
            SG = rp.tile([NB, G4], BF16, tag="SG")
            nc.scalar.activation(SG[:, 512:1024], Gt[1][:], AF.Sigmoid)   # f
            nc.scalar.activation(SG[:, 0:512], Gt[0][:], AF.Sigmoid)     # i
            nc.scalar.activation(SG[:, 1536:2048], Gt[3][:], AF.Tanh)    # g
            sel_next = []
            if t + 1 < S:
                Gnext, sel_next = emit_sel(t + 1)
            t1 = rp.tile([NB, H], BF16, tag="t1")
            t2 = rp.tile([NB, H], BF16, tag="t2")
            th = rp.tile([NB, H], BF16, tag="th")
            h = rp.tile([NB, H], BF16, tag="h")
            nc.vector.tensor_mul(t1[:], SG[:, 512:1024], cprev[:])
            nc.vector.tensor_mul(t2[:], SG[:, 0:512], SG[:, 1536:2048])
            nc.vector.tensor_add(cnext[:], t1[:], t2[:])
            nc.scalar.activation(th[:], cnext[:], AF.Tanh)
            # tail: chunked o-sigmoid / h / transpose / hsT copy so next-step
            # matmuls restart as soon as the first hsT half lands
            HT = ps_g.tile([128, 4 * NB], BF16, tag="G")
            hsT_v = hsT.rearrange("p (c n) -> p c n", c=4)
            HT_v = HT.rearrange("p (c n) -> p c n", c=4)
            for cc in range(2):
                sl = slice(256 * cc, 256 * cc + 256)
                nc.scalar.activation(SG[:, 1024 + 256 * cc: 1024 + 256 * cc + 256],
                                     Gt[2][:, sl], AF.Sigmoid)             # o half
                nc.vector.tensor_mul(h[:, sl], SG[:, 1024 + 256 * cc: 1024 + 256 * cc + 256], th[:, sl])
                for kc in (2 * cc, 2 * cc + 1):
                    tr = nc.tensor.transpose(HT[:, 8 * kc: 8 * kc + 8], h[:, 128 * kc: 128 * kc + 128], id8[:])
                    for si in sel_next:
                        add_dep_helper(tr.ins, si, sync=False,
                                       reason="order next-step x-gate MMs into the tail bubble")
                nc.vector.tensor_copy(hsT_v[:, 2 * cc: 2 * cc + 2, 8 * t: 8 * t + 8],
                                      HT_v[:, 2 * cc: 2 * cc + 2, :])
            cprev, cnext = cnext, cprev


def _phase_em(nc, tc, dp, hsT, ones128, emT, d):
    """Emissions (bt,45) -> per-core keep/swap permutation -> AllReduce -> emT (45,N)."""
    with tc.tile_pool(name="em", bufs=1) as ep, \
         tc.tile_pool(name="ps_em", bufs=2, space="PSUM") as ps_em:
        lin45 = ep.tile([128, 4 * T], BF16)
        linb1 = ep.tile([1, T], F32)
        keep = ep.tile([128, 128], F32)
        swap = ep.tile([128, 128], F32)
        id128b = ep.tile([128, 128], F32)
        emA = ep.tile([128, 16 * T], F32)
        emB = ep.tile([128, 16 * T], F32)
        emC = ep.tile([128, 16 * T], F32)
        nc.sync.dma_start(out=lin45[:], in_=d["lin45"][:])
        nc.sync.dma_start(out=linb1[:], in_=d["linb1"][:])
        nc.sync.dma_start(out=keep[:], in_=d["keepM"][:])
        nc.sync.dma_start(out=swap[:], in_=d["swapM"][:])
        nc.sync.dma_start(out=id128b[:], in_=d["id128"][:])
        for j in range(16):
            pe = ps_em.tile([128, T], F32, tag="pe")
            nc.tensor.matmul(pe[:], ones128[:], linb1[:], start=True, stop=False)
            for kc in range(4):
                nc.tensor.matmul(
                    pe[:], hsT[:, G4 * kc + 128 * j: G4 * kc + 128 * j + 128],
                    lin45[:, T * kc: T * kc + T], start=False, stop=(kc == 3))
            nc.vector.tensor_copy(emA[:, T * j: T * (j + 1)], pe[:])
        for j in range(16):
            pb = ps_em.tile([128, T], F32, tag="pb")
            nc.tensor.matmul(pb[:], keep[:], emA[:, T * j: T * (j + 1)], start=True, stop=False)
            nc.tensor.matmul(pb[:], swap[:], emA[:, T * (15 - j): T * (16 - j)], start=False, stop=True)
            nc.vector.tensor_copy(emB[:, T * j: T * (j + 1)], pb[:])

        bounceA = dp.tile([N, T], F32)
        bounceB = dp.tile([N, T], F32)
        # DRAM-side AP iterating (p, j, k) to match SBUF (partition, j, k)
        dstA = bass.AP(bounceA.tensor, 0, [[T, 128], [T * 128, 16], [1, T]])
        nc.sync.dma_start(out=dstA, in_=emB.rearrange("p (j k) -> p j k", j=16))
        nc.gpsimd.collective_compute(
            "AllReduce", ALU.add,
            replica_groups=[[0, 4], [1, 5], [2, 6], [3, 7]],
            ins=[bounceA.opt()], outs=[bounceB.opt()],
        )
        srcB = bass.AP(bounceB.tensor, 0, [[T, 128], [T * 128, 16], [1, T]])
        nc.sync.dma_start(out=emC.rearrange("p (j k) -> p j k", j=16), in_=srcB)
        for j in range(16):
            pt = ps_em.tile([T, 128], F32, tag="pt")
            nc.tensor.transpose(pt[:], emC[:, T * j: T * (j + 1)], id128b[:])
            nc.vector.tensor_copy(emT[:, 128 * j: 128 * (j + 1)], pt[:])


def _build(stop_after=None):
    lv = {"B": 1, "rec": 2, "em": 3, "crf": 4, None: 5}[stop_after]
    nc = bacc.Bacc("TRN2", target_bir_lowering=False, debug=False, num_devices=8)

    d = {}
    d["emb"] = nc.dram_tensor("emb", [V, E], F32, kind="ExternalInput")
    d["xidx"] = nc.dram_tensor("xidx", [128, 16], I32, kind="ExternalInput")
    d["wihT"] = nc.dram_tensor("wihT", [128, 2 * G4], F32, kind="ExternalInput")
    d["bias1"] = nc.dram_tensor("bias1", [1, G4], F32, kind="ExternalInput")
    d["whhT"] = nc.dram_tensor("whhT", [128, 4 * G4], BF16, kind="ExternalInput")
    d["sel"] = nc.dram_tensor("sel", [128, 128], BF16, kind="ExternalInput")
    d["id8"] = nc.dram_tensor("id8", [8, 8], BF16, kind="ExternalInput")
    d["id128"] = nc.dram_tensor("id128", [128, 128], F32, kind="ExternalInput")
    d["lin45"] = nc.dram_tensor("lin45", [128, 4 * T], BF16, kind="ExternalInput")
    d["linb1"] = nc.dram_tensor("linb1", [1, T], F32, kind="ExternalInput")
    d["keepM"] = nc.dram_tensor("keepM", [128, 128], F32, kind="ExternalInput")
    d["swapM"] = nc.dram_tensor("swapM", [128, 128], F32, kind="ExternalInput")
    d["trans"] = nc.dram_tensor("trans", [T, T], F32, kind="ExternalInput")
    d["stend"] = nc.dram_tensor("stend", [T, 2], F32, kind="ExternalInput")
    d["oh"] = nc.dram_tensor("oh", [T, N], F32, kind="ExternalInput")
    d["oh2"] = nc.dram_tensor("oh2", [T, N], F32, kind="ExternalInput")
    d_loss = nc.dram_tensor("loss", [1, NB], F32, kind="ExternalOutput")

    with tile.TileContext(nc) as tc:
        with tc.tile_pool(name="persist", bufs=1) as pp, \
             tc.tile_pool(name="dram", bufs=1, space="DRAM") as dp:
            gates_x = pp.tile([128, 16 * G4], BF16)
            whhT = pp.tile([128, 4 * G4], BF16)
            hsT = pp.tile([128, 4 * G4], BF16)
            selb = pp.tile([128, 128], BF16)
            id8 = pp.tile([8, 8], BF16)
            ones128 = pp.tile([1, 128], F32)
            c0 = pp.tile([8, H], BF16)
            c1 = pp.tile([8, H], BF16)

            nc.sync.dma_start(out=whhT[:], in_=d["whhT"][:])
            nc.sync.dma_start(out=selb[:], in_=d["sel"][:])
            nc.sync.dma_start(out=id8[:], in_=d["id8"][:])
            nc.vector.memset(ones128[:], 1.0)
            nc.vector.memset(c0[:], 0.0)

            _phase_ab(nc, tc, gates_x, ones128, d)
            if lv == 1:
                nc.sync.dma_start(out=d_loss[:], in_=ones128[:, 0:NB])

            if lv >= 2:
                _phase_rec(nc, tc, gates_x, whhT, hsT, selb, id8, c0, c1)
                if lv == 2:
                    nc.gpsimd.dma_start(out=d_loss[:], in_=hsT[0:1, 0:NB])

            emT = pp.tile([T, N], F32)
            if lv >= 3:
                _phase_em(nc, tc, dp, hsT, ones128, emT, d)
                if lv == 3:
                    nc.sync.dma_start(out=d_loss[:], in_=emT[0:1, 0:NB])

            if lv >= 4:
                with tc.tile_pool(name="crf", bufs=1) as cp, \
                     tc.tile_pool(name="ps_q", bufs=2, space="PSUM") as ps_q, \
                     tc.tile_pool(name="qs", bufs=2) as qp:
                    exp_em = cp.tile([T, N], F32)
                    trans_sb = cp.tile([T, T], F32)
                    stend = cp.tile([T, 2], F32)
                    Ep = cp.tile([T, T], F32)
                    estart = cp.tile([T, 1], F32)
                    eend = cp.tile([T, 1], F32)
                    ones45 = cp.tile([T, 1], F32)
                    nln45 = cp.tile([T, 1], F32)
                    nc.vector.memset(nln45[:], -LN45)
                    oh = cp.tile([T, N], F32)
                    oh2 = cp.tile([T, N], F32)
                    nc.sync.dma_start(out=oh2[:], in_=d["oh2"][:])
                    nc.sync.dma_start(out=trans_sb[:], in_=d["trans"][:])
                    nc.sync.dma_start(out=stend[:], in_=d["stend"][:])
                    nc.sync.dma_start(out=oh[:], in_=d["oh"][:])
                    nc.vector.memset(ones45[:], 1.0)
                    nc.scalar.activation(exp_em[:], emT[:], AF.Exp)
                    nc.scalar.activation(Ep[:], trans_sb[:], AF.Exp, bias=nln45[:])
                    nc.scalar.activation(estart[:], stend[:, 0:1], AF.Exp)
                    nc.scalar.activation(eend[:], stend[:, 1:2], AF.Exp)

                    # two interleaved alpha chains (seqs 0-3 / 4-7) hide the
                    # PE<->DVE round-trip latency of the per-step recursion
                    HB = NB // 2
                    qA = qp.tile([T, HB], F32, tag="qA")
                    qB = qp.tile([T, HB], F32, tag="qB")
                    nc.vector.tensor_scalar_mul(qA[:], exp_em[:, 0:HB], estart[:])
                    nc.vector.tensor_scalar_mul(qB[:], exp_em[:, HB:NB], estart[:])
                    for t in range(1, S):
                        sA = ps_q.tile([T, HB], F32, tag="sA")
                        nc.tensor.matmul(sA[:], Ep[:], qA[:], start=True, stop=True)
                        sB = ps_q.tile([T, HB], F32, tag="sB")
                        nc.tensor.matmul(sB[:], Ep[:], qB[:], start=True, stop=True)
                        qAn = qp.tile([T, HB], F32, tag="qA")
                        nc.vector.tensor_mul(qAn[:], sA[:], exp_em[:, NB * t: NB * t + HB])
                        qBn = qp.tile([T, HB], F32, tag="qB")
                        nc.vector.tensor_mul(qBn[:], sB[:], exp_em[:, NB * t + HB: NB * (t + 1)])
                        qA, qB = qAn, qBn
                    if lv == 4:
                        nc.sync.dma_start(out=d_loss[:], in_=qA[0:1, :].to_broadcast([1, NB]))

                    if lv >= 5:
                        w = cp.tile([T, NB], F32)
                        nc.vector.tensor_scalar_mul(w[:, 0:HB], qA[:], eend[:])
                        nc.vector.tensor_scalar_mul(w[:, HB:NB], qB[:], eend[:])
                        logZ = cp.tile([1, NB], F32)
                        em_sc = cp.tile([1, NB], F32)
                        tr_sc = cp.tile([1, NB], F32)
                        sten_s = cp.tile([1, NB], F32)
                        with tc.tile_pool(name="ps_f1", bufs=1, space="PSUM") as ps_f1:
                            sumw = ps_f1.tile([1, NB], F32, tag="f1")
                            nc.tensor.matmul(sumw[:], ones45[:], w[:], start=True, stop=True)
                            nc.scalar.activation(logZ[:], sumw[:], AF.Ln)
                        with tc.tile_pool(name="ps_s1", bufs=1, space="PSUM") as ps_s1:
                            S1 = cp.tile([T, N], F32)
                            nc.vector.tensor_mul(S1[:], emT[:], oh[:])
                            s1p = ps_s1.tile([1, N], F32, tag="fbig")
                            for ck in range(4):
                                nc.tensor.matmul(s1p[:, 512 * ck: 512 * ck + 512], ones45[:],
                                                 S1[:, 512 * ck: 512 * ck + 512], start=True, stop=True)
                            nc.vector.tensor_reduce(
                                em_sc[:], s1p.rearrange("p (t b) -> p b t", b=NB),
                                axis=mybir.AxisListType.X, op=ALU.add)
                        with tc.tile_pool(name="ps_R", bufs=1, space="PSUM") as ps_R:
                            Rp_ = ps_R.tile([T, N], F32, tag="fR")
                            for ck in range(4):
                                nc.tensor.matmul(Rp_[:, 512 * ck: 512 * ck + 512], trans_sb[:],
                                                 oh[:, 512 * ck: 512 * ck + 512], start=True, stop=True)
                            S2 = cp.tile([T, N], F32)
                            nc.vector.tensor_mul(S2[:], Rp_[:], oh2[:])
                        with tc.tile_pool(name="ps_s2", bufs=1, space="PSUM") as ps_s2:
                            s2p = ps_s2.tile([1, N], F32, tag="fbig")
                            for ck in range(4):
                                nc.tensor.matmul(s2p[:, 512 * ck: 512 * ck + 512], ones45[:],
                                                 S2[:, 512 * ck: 512 * ck + 512], start=True, stop=True)
                            nc.vector.tensor_reduce(
                                tr_sc[:], s2p.rearrange("p (t b) -> p b t", b=NB),
                                axis=mybir.AxisListType.X, op=ALU.add)
                        with tc.tile_pool(name="ps_f2", bufs=1, space="PSUM") as ps_f2:
                            stp = cp.tile([T, NB], F32)
                            enp = cp.tile([T, NB], F32)
                            nc.vector.tensor_scalar_mul(stp[:], oh[:, 0:NB], stend[:, 0:1])
                            nc.vector.tensor_scalar_mul(enp[:], oh[:, NB * (S - 1):N], stend[:, 1:2])
                            sten = ps_f2.tile([1, NB], F32, tag="f2")
                            nc.tensor.matmul(sten[:], ones45[:], stp[:], start=True, stop=False)
                            nc.tensor.matmul(sten[:], ones45[:], enp[:], start=False, stop=True)
                            nc.vector.tensor_copy(sten_s[:], sten[:])

                        sc1 = cp.tile([1, NB], F32)
                        sc2 = cp.tile([1, NB], F32)
                        lossa = cp.tile([1, NB], F32)
                        lossb = cp.tile([1, NB], F32)
                        nc.vector.tensor_add(sc1[:], em_sc[:], tr_sc[:])
                        nc.vector.tensor_add(sc2[:], sc1[:], sten_s[:])
                        nc.vector.tensor_tensor(out=lossa[:], in0=logZ[:], in1=sc2[:], op=ALU.subtract)
                        nc.scalar.activation(lossb[:], lossa[:], AF.Copy, bias=(S - 1) * LN45)
                        nc.sync.dma_start(out=d_loss[:], in_=lossb[:])

    nc.finalize()
    return nc


def _perm_gates(w):
    i, f, g, o = np.split(w, 4, axis=0)
    return np.concatenate([i, f, o, g], axis=0)


def _pack_k(wT, kchunks):
    # [K, M] -> [128, kchunks*M] with k-chunk kc at col block kc
    K, M = wT.shape
    assert K == 128 * kchunks
    return np.ascontiguousarray(wT.reshape(kchunks, 128, M).transpose(1, 0, 2).reshape(128, kchunks * M))


def prepare_in_maps(**inputs):
    x = np.asarray(inputs["x"]).astype(np.int32)          # [32, 256]
    tags = np.asarray(inputs["tags"]).astype(np.int32)
    emb = np.asarray(inputs["emb"], dtype=np.float32)
    lin_w = np.asarray(inputs["lin_w"], dtype=np.float32)
    lin_b = np.asarray(inputs["lin_b"], dtype=np.float32)
    start_t = np.asarray(inputs["start_t"], dtype=np.float32)
    end_t = np.asarray(inputs["end_t"], dtype=np.float32)
    trans = np.asarray(inputs["trans"], dtype=np.float32)

    wih = {0: _perm_gates(np.asarray(inputs["w_ih_f"], np.float32)),
           1: _perm_gates(np.asarray(inputs["w_ih_b"], np.float32))}
    whh = {0: _perm_gates(np.asarray(inputs["w_hh_f"], np.float32)),
           1: _perm_gates(np.asarray(inputs["w_hh_b"], np.float32))}
    bb = {0: _perm_gates(np.asarray(inputs["b_f"], np.float32)),
          1: _perm_gates(np.asarray(inputs["b_b"], np.float32))}

    id128 = np.eye(128, dtype=np.float32)
    id8 = np.eye(8, dtype=np.float32)
    u = np.arange(128)
    rev = 8 * (15 - (u // 8)) + (u % 8)
    revM = np.zeros((128, 128), np.float32)
    revM[u, rev[u]] = 1.0
    sel = np.eye(128, dtype=np.float32)   # col 8u+b selects row 8u+b

    in_maps = []
    for core in range(8):
        dirn, grp = core // 4, core % 4
        xs = x[8 * grp: 8 * grp + 8]
        if dirn == 1:
            xs = xs[:, ::-1]
        x_flat = xs.T.reshape(-1)                         # n = 8t+b
        xidx = np.ascontiguousarray(x_flat.reshape(16, 128).T).astype(np.int32)

        tg = tags[8 * grp: 8 * grp + 8]                   # [8, 256]
        oh = np.zeros((T, N), np.float32)
        oh[tg.T.reshape(-1), np.arange(N)] = 1.0
        oh2c = np.zeros((T, N), np.float32)
        oh2c[:, 0:N - NB] = oh[:, NB:N]

        lin_half = lin_w[:, 512 * dirn: 512 * (dirn + 1)]  # [45, 512]
        in_maps.append({
            "emb": emb,
            "xidx": xidx,
            "wihT": _pack_k(wih[dirn].T.copy(), 2),
            "bias1": bb[dirn].reshape(1, G4),
            "whhT": _pack_k(whh[dirn].T.copy(), 4).astype(ml_dtypes.bfloat16),
            "sel": sel.astype(ml_dtypes.bfloat16),
            "id8": id8.astype(ml_dtypes.bfloat16),
            "id128": id128,
            "lin45": _pack_k(np.ascontiguousarray(lin_half.T), 4).astype(ml_dtypes.bfloat16),
            "linb1": (lin_b if dirn == 0 else np.zeros_like(lin_b)).reshape(1, T),
            "keepM": id128 if dirn == 0 else np.zeros_like(id128),
            "swapM": np.zeros_like(revM) if dirn == 0 else revM,
            "trans": trans,
            "stend": np.stack([start_t, end_t], axis=1),
            "oh": oh if dirn == 0 else np.zeros_like(oh),
            "oh2": oh2c if dirn == 0 else np.zeros_like(oh),
        })

    return in_maps


def get_nc():
    if "nc" not in _cached:
        _cached["nc"] = _build()
    return _cached["nc"]


def kernel(**inputs):
    in_maps = prepare_in_maps(**inputs)
    res = run_bass_kernel_spmd(get_nc(), in_maps, core_ids=list(range(8)))
    total = np.float64(0.0)
    for core in range(4):
        total += np.float64(res.results[core]["loss"]).sum()
    return np.float32(total / 32.0)


# revision 27
# speedup vs baseline: 41.7472x; 1.0652x over previous
"""BiLSTM-CRF NLL loss on 8 Trainium2 NeuronCores.

Sharding: core c in 0..7 -> direction = c//4 (0=fwd, 1=bwd on time-reversed
input), batch group = c%4 (8 sequences each). Each core: embedding gather ->
input transform -> LSTM recurrence (256 steps) -> partial emissions ->
pairwise AllReduce (fwd+bwd emission halves) -> exp-domain CRF forward
algorithm + gold-path score -> per-core loss partials. Host sums partials/32.

Self-contained: hardcodes all shapes; only needs numpy + concourse (+ml_dtypes).
"""
import numpy as np
import ml_dtypes

import concourse.bass as bass
import concourse.bacc as bacc
import concourse.tile as tile
from concourse.tile_rust import add_dep_helper
from concourse import mybir
from concourse.bass_utils import run_bass_kernel_spmd

F32 = mybir.dt.float32
BF16 = mybir.dt.bfloat16
I32 = mybir.dt.int32
AF = mybir.ActivationFunctionType
ALU = mybir.AluOpType

B, S, E, H, T, V = 32, 256, 256, 512, 45, 50000
G4 = 4 * H          # 2048 gates
NB = 8              # sequences per core
N = S * NB          # 2048 rows, t-major: n = 8t+b
LN45 = float(np.log(45.0))

_cached = {}


def _ab_setup(nc, ab, d):
    ctx = {
        "X": ab.tile([128, 16 * E], F32, tag="abX", name="abX"),
        "XT": ab.tile([128, 2 * 2048], F32, tag="abXT", name="abXT"),
        "wihT": ab.tile([128, 2 * G4], F32, tag="abWih", name="abWih"),
        "bias1": ab.tile([1, G4], F32, tag="abB", name="abB"),
        "xidx": ab.tile([128, 16], I32, tag="abIdx", name="abIdx"),
        "id128": ab.tile([128, 128], F32, tag="abId", name="abId"),
    }
    nc.sync.dma_start(out=ctx["wihT"][:], in_=d["wihT"][:])
    nc.sync.dma_start(out=ctx["bias1"][:], in_=d["bias1"][:])
    nc.sync.dma_start(out=ctx["xidx"][:], in_=d["xidx"][:])
    nc.sync.dma_start(out=ctx["id128"][:], in_=d["id128"][:])
    return ctx


def _ab_block(nc, j, ctx, gates_x, ones128, ps_tp, ps_gx, d):
    """Gather + transpose + input transform for one 16-step block j."""
    X, XT = ctx["X"], ctx["XT"]
    nc.gpsimd.indirect_dma_start(
        out=X[:, 256 * j: 256 * j + 256],
        out_offset=None,
        in_=d["emb"][:],
        in_offset=bass.IndirectOffsetOnAxis(ap=ctx["xidx"][:, j:j + 1], axis=0),
    )
    for ec in range(2):
        tp = ps_tp.tile([128, 128], F32, tag="tp")
        nc.tensor.transpose(tp[:], X[:, 256 * j + 128 * ec: 256 * j + 128 * ec + 128], ctx["id128"][:])
        nc.vector.tensor_copy(XT[:, 2048 * ec + 128 * j: 2048 * ec + 128 * j + 128], tp[:])
    for ng in range(4):
        gx = ps_gx.tile([128, 512], F32, tag="gx")
        nc.tensor.matmul(gx[:], ones128[:], ctx["bias1"][:, 512 * ng: 512 * ng + 512], start=True, stop=False)
        for ec in range(2):
            nc.tensor.matmul(
                gx[:],
                XT[:, 2048 * ec + 128 * j: 2048 * ec + 128 * j + 128],
                ctx["wihT"][:, G4 * ec + 512 * ng: G4 * ec + 512 * ng + 512],
                start=False, stop=(ec == 1))
        nc.vector.tensor_copy(gates_x[:, G4 * j + 512 * ng: G4 * j + 512 * ng + 512], gx[:])


def _phase_rec(nc, tc, gates_x, whhT, hsT, selb, id8, c0, c1, emit_ab=None):
    """LSTM recurrence, 256 steps; writes hsT (h transposed, col 8t+b).

    Gate order in packed weights: [i, f, o, g]. Per-step: 4 independent PSUM
    gate tiles (f first) -> per-gate activations -> bf16 cell update ->
    PE transpose of h -> hsT. bf16 datapath for 2x DVE and 2x PE streaming.
    emit_ab(j) streams the embedding-gather + input-transform for block j
    into the loop so the serial gpsimd gathers overlap the recurrence.
    """
    with tc.tile_pool(name="rec", bufs=3) as rp, \
         tc.tile_pool(name="ps_g", bufs=6, space="PSUM") as ps_g:
        cprev, cnext = c0, c1
        NGORD = (1, 0, 3, 2)   # f, i, g, o

        def emit_sel(t):
            # inject x-gates for step t (start=True opens each PSUM bank);
            # hoisted to fill the PE bubble while step t-1's tail drains
            m, u = t // 16, t % 16
            Gt = {}
            sel_ins = []
            for ng in NGORD:
                Gn = ps_g.tile([NB, 512], F32, tag="G")
                Gt[ng] = Gn
                mm = nc.tensor.matmul(
                    Gn[:], selb[:, 8 * u: 8 * u + 8],
                    gates_x[:, G4 * m + 512 * ng: G4 * m + 512 * ng + 512],
                    start=True, stop=(t == 0))
                sel_ins.append(mm.ins)
            return Gt, sel_ins

        Gnext, _ = emit_sel(0)
        for t in range(S):
            Gt = Gnext
            if emit_ab is not None and t % 16 == 0 and t // 16 + 2 < 16:
                emit_ab(t // 16 + 2)
            if t > 0:
                for ng in NGORD:
                    for kc in range(4):
                        nc.tensor.matmul(
                            Gt[ng][:], hsT[:, G4 * kc + 8 * (t - 1): G4 * kc + 8 * (t - 1) + 8],
                            whhT[:, G4 * kc + 512 * ng: G4 * kc + 512 * ng + 512],
                            start=False, stop=(kc == 3))
            SG = rp.tile([NB, G4], BF16, tag="SG")
            nc.scalar.activation(SG[:, 512:1024], Gt[1][:], AF.Sigmoid)   # f
            nc.scalar.activation(SG[:, 0:512], Gt[0][:], AF.Sigmoid)     # i
            nc.scalar.activation(SG[:, 1536:2048], Gt[3][:], AF.Tanh)    # g
            sel_next = []
            if t + 1 < S:
                Gnext, sel_next = emit_sel(t + 1)
            t1 = rp.tile([NB, H], BF16, tag="t1")
            t2 = rp.tile([NB, H], BF16, tag="t2")
            th = rp.tile([NB, H], BF16, tag="th")
            h = rp.tile([NB, H], BF16, tag="h")
            nc.vector.tensor_mul(t1[:], SG[:, 512:1024], cprev[:])
            nc.vector.tensor_mul(t2[:], SG[:, 0:512], SG[:, 1536:2048])
            nc.vector.tensor_add(cnext[:], t1[:], t2[:])
            nc.scalar.activation(th[:], cnext[:], AF.Tanh)
            # tail: chunked o-sigmoid / h / transpose / hsT copy so next-step
            # matmuls restart as soon as the first hsT half lands
            HT = ps_g.tile([128, 4 * NB], BF16, tag="G")
            hsT_v = hsT.rearrange("p (c n) -> p c n", c=4)
            HT_v = HT.rearrange("p (c n) -> p c n", c=4)
            for cc in range(2):
                sl = slice(256 * cc, 256 * cc + 256)
                nc.scalar.activation(SG[:, 1024 + 256 * cc: 1024 + 256 * cc + 256],
                                     Gt[2][:, sl], AF.Sigmoid)             # o half
                nc.vector.tensor_mul(h[:, sl], SG[:, 1024 + 256 * cc: 1024 + 256 * cc + 256], th[:, sl])
                for kc in (2 * cc, 2 * cc + 1):
                    tr = nc.tensor.transpose(HT[:, 8 * kc: 8 * kc + 8], h[:, 128 * kc: 128 * kc + 128], id8[:])
                    for si in sel_next:
                        add_dep_helper(tr.ins, si, sync=False,
                                       reason="order next-step x-gate MMs into the tail bubble")
                nc.vector.tensor_copy(hsT_v[:, 2 * cc: 2 * cc + 2, 8 * t: 8 * t + 8],
                                      HT_v[:, 2 * cc: 2 * cc + 2, :])
            cprev, cnext = cnext, cprev


def _phase_em(nc, tc, dp, hsT, ones128, emT, d):
    """Emissions (bt,45) -> per-core keep/swap permutation -> AllReduce -> emT (45,N)."""
    with tc.tile_pool(name="em", bufs=1) as ep, \
         tc.tile_pool(name="ps_em", bufs=2, space="PSUM") as ps_em:
        lin45 = ep.tile([128, 4 * T], BF16)
        linb1 = ep.tile([1, T], F32)
        keep = ep.tile([128, 128], F32)
        swap = ep.tile([128, 128], F32)
        id128b = ep.tile([128, 128], F32)
        emA = ep.tile([128, 16 * T], F32)
        emB = ep.tile([128, 16 * T], F32)
        emC = ep.tile([128, 16 * T], F32)
        nc.sync.dma_start(out=lin45[:], in_=d["lin45"][:])
        nc.sync.dma_start(out=linb1[:], in_=d["linb1"][:])
        nc.sync.dma_start(out=keep[:], in_=d["keepM"][:])
        nc.sync.dma_start(out=swap[:], in_=d["swapM"][:])
        nc.sync.dma_start(out=id128b[:], in_=d["id128"][:])
        for j in range(16):
            pe = ps_em.tile([128, T], F32, tag="pe")
            nc.tensor.matmul(pe[:], ones128[:], linb1[:], start=True, stop=False)
            for kc in range(4):
                nc.tensor.matmul(
                    pe[:], hsT[:, G4 * kc + 128 * j: G4 * kc + 128 * j + 128],
                    lin45[:, T * kc: T * kc + T], start=False, stop=(kc == 3))
            nc.vector.tensor_copy(emA[:, T * j: T * (j + 1)], pe[:])
        for j in range(16):
            pb = ps_em.tile([128, T], F32, tag="pb")
            nc.tensor.matmul(pb[:], keep[:], emA[:, T * j: T * (j + 1)], start=True, stop=False)
            nc.tensor.matmul(pb[:], swap[:], emA[:, T * (15 - j): T * (16 - j)], start=False, stop=True)
            nc.vector.tensor_copy(emB[:, T * j: T * (j + 1)], pb[:])

        bounceA = dp.tile([N, T], F32)
        bounceB = dp.tile([N, T], F32)
        # DRAM-side AP iterating (p, j, k) to match SBUF (partition, j, k)
        dstA = bass.AP(bounceA.tensor, 0, [[T, 128], [T * 128, 16], [1, T]])
        nc.sync.dma_start(out=dstA, in_=emB.rearrange("p (j k) -> p j k", j=16))
        nc.gpsimd.collective_compute(
            "AllReduce", ALU.add,
            replica_groups=[[0, 4], [1, 5], [2, 6], [3, 7]],
            ins=[bounceA.opt()], outs=[bounceB.opt()],
        )
        srcB = bass.AP(bounceB.tensor, 0, [[T, 128], [T * 128, 16], [1, T]])
        nc.sync.dma_start(out=emC.rearrange("p (j k) -> p j k", j=16), in_=srcB)
        for j in range(16):
            pt = ps_em.tile([T, 128], F32, tag="pt")
            nc.tensor.transpose(pt[:], emC[:, T * j: T * (j + 1)], id128b[:])
            nc.vector.tensor_copy(emT[:, 128 * j: 128 * (j + 1)], pt[:])


def _build(stop_after=None):
    lv = {"B": 1, "rec": 2, "em": 3, "crf": 4, None: 5}[stop_after]
    nc = bacc.Bacc("TRN2", target_bir_lowering=False, debug=False, num_devices=8)

    d = {}
    d["emb"] = nc.dram_tensor("emb", [V, E], F32, kind="ExternalInput")
    d["xidx"] = nc.dram_tensor("xidx", [128, 16], I32, kind="ExternalInput")
    d["wihT"] = nc.dram_tensor("wihT", [128, 2 * G4], F32, kind="ExternalInput")
    d["bias1"] = nc.dram_tensor("bias1", [1, G4], F32, kind="ExternalInput")
    d["whhT"] = nc.dram_tensor("whhT", [128, 4 * G4], BF16, kind="ExternalInput")
    d["sel"] = nc.dram_tensor("sel", [128, 128], BF16, kind="ExternalInput")
    d["id8"] = nc.dram_tensor("id8", [8, 8], BF16, kind="ExternalInput")
    d["id128"] = nc.dram_tensor("id128", [128, 128], F32, kind="ExternalInput")
    d["lin45"] = nc.dram_tensor("lin45", [128, 4 * T], BF16, kind="ExternalInput")
    d["linb1"] = nc.dram_tensor("linb1", [1, T], F32, kind="ExternalInput")
    d["keepM"] = nc.dram_tensor("keepM", [128, 128], F32, kind="ExternalInput")
    d["swapM"] = nc.dram_tensor("swapM", [128, 128], F32, kind="ExternalInput")
    d["trans"] = nc.dram_tensor("trans", [T, T], F32, kind="ExternalInput")
    d["stend"] = nc.dram_tensor("stend", [T, 2], F32, kind="ExternalInput")
    d["oh"] = nc.dram_tensor("oh", [T, N], F32, kind="ExternalInput")
    d["oh2"] = nc.dram_tensor("oh2", [T, N], F32, kind="ExternalInput")
    d_loss = nc.dram_tensor("loss", [1, NB], F32, kind="ExternalOutput")

    with tile.TileContext(nc) as tc:
        with tc.tile_pool(name="persist", bufs=1) as pp, \
             tc.tile_pool(name="dram", bufs=1, space="DRAM") as dp:
            gates_x = pp.tile([128, 16 * G4], BF16)
            whhT = pp.tile([128, 4 * G4], BF16)
            hsT = pp.tile([128, 4 * G4], BF16)
            selb = pp.tile([128, 128], BF16)
            id8 = pp.tile([8, 8], BF16)
            ones128 = pp.tile([1, 128], F32)
            c0 = pp.tile([8, H], BF16)
            c1 = pp.tile([8, H], BF16)

            nc.sync.dma_start(out=whhT[:], in_=d["whhT"][:])
            nc.sync.dma_start(out=selb[:], in_=d["sel"][:])
            nc.sync.dma_start(out=id8[:], in_=d["id8"][:])
            nc.vector.memset(ones128[:], 1.0)
            nc.vector.memset(c0[:], 0.0)

            with tc.tile_pool(name="ab", bufs=1) as ab, \
                 tc.tile_pool(name="ps_tp", bufs=1, space="PSUM") as ps_tp, \
                 tc.tile_pool(name="ps_gx", bufs=1, space="PSUM") as ps_gx:
                ctx = _ab_setup(nc, ab, d)

                def emit_ab(j):
                    _ab_block(nc, j, ctx, gates_x, ones128, ps_tp, ps_gx, d)

                emit_ab(0)
                emit_ab(1)
                if lv == 1:
                    for j in range(2, 16):
                        emit_ab(j)
                    nc.sync.dma_start(out=d_loss[:], in_=ones128[:, 0:NB])
                if lv >= 2:
                    _phase_rec(nc, tc, gates_x, whhT, hsT, selb, id8, c0, c1,
                               emit_ab=emit_ab)
                    if lv == 2:
                        nc.gpsimd.dma_start(out=d_loss[:], in_=hsT[0:1, 0:NB])

            emT = pp.tile([T, N], F32)
            if lv >= 3:
                _phase_em(nc, tc, dp, hsT, ones128, emT, d)
                if lv == 3:
                    nc.sync.dma_start(out=d_loss[:], in_=emT[0:1, 0:NB])

            if lv >= 4:
                with tc.tile_pool(name="crf", bufs=1) as cp, \
                     tc.tile_pool(name="ps_q", bufs=2, space="PSUM") as ps_q, \
                     tc.tile_pool(name="qs", bufs=2) as qp:
                    exp_em = cp.tile([T, N], F32)
                    trans_sb = cp.tile([T, T], F32)
                    stend = cp.tile([T, 2], F32)
                    Ep = cp.tile([T, T], F32)
                    estart = cp.tile([T, 1], F32)
                    eend = cp.tile([T, 1], F32)
                    ones45 = cp.tile([T, 1], F32)
                    nln45 = cp.tile([T, 1], F32)
                    nc.vector.memset(nln45[:], -LN45)
                    oh = cp.tile([T, N], F32)
                    oh2 = cp.tile([T, N], F32)
                    nc.sync.dma_start(out=oh2[:], in_=d["oh2"][:])
                    nc.sync.dma_start(out=trans_sb[:], in_=d["trans"][:])
                    nc.sync.dma_start(out=stend[:], in_=d["stend"][:])
                    nc.sync.dma_start(out=oh[:], in_=d["oh"][:])
                    nc.vector.memset(ones45[:], 1.0)
                    nc.scalar.activation(exp_em[:], emT[:], AF.Exp)
                    nc.scalar.activation(Ep[:], trans_sb[:], AF.Exp, bias=nln45[:])
                    nc.scalar.activation(estart[:], stend[:, 0:1], AF.Exp)
                    nc.scalar.activation(eend[:], stend[:, 1:2], AF.Exp)

                    # two interleaved alpha chains (seqs 0-3 / 4-7) hide the
                    # PE<->DVE round-trip latency of the per-step recursion
                    HB = NB // 2
                    qA = qp.tile([T, HB], F32, tag="qA")
                    qB = qp.tile([T, HB], F32, tag="qB")
                    nc.vector.tensor_scalar_mul(qA[:], exp_em[:, 0:HB], estart[:])
                    nc.vector.tensor_scalar_mul(qB[:], exp_em[:, HB:NB], estart[:])
                    for t in range(1, S):
                        sA = ps_q.tile([T, HB], F32, tag="sA")
                        nc.tensor.matmul(sA[:], Ep[:], qA[:], start=True, stop=True)
                        sB = ps_q.tile([T, HB], F32, tag="sB")
                        nc.tensor.matmul(sB[:], Ep[:], qB[:], start=True, stop=True)
                        qAn = qp.tile([T, HB], F32, tag="qA")
                        nc.vector.tensor_mul(qAn[:], sA[:], exp_em[:, NB * t: NB * t + HB])
                        qBn = qp.tile([T, HB], F32, tag="qB")
                        nc.vector.tensor_mul(qBn[:], sB[:], exp_em[:, NB * t + HB: NB * (t + 1)])
                        qA, qB = qAn, qBn
                    if lv == 4:
                        nc.sync.dma_start(out=d_loss[:], in_=qA[0:1, :].to_broadcast([1, NB]))

                    if lv >= 5:
                        w = cp.tile([T, NB], F32)
                        nc.vector.tensor_scalar_mul(w[:, 0:HB], qA[:], eend[:])
                        nc.vector.tensor_scalar_mul(w[:, HB:NB], qB[:], eend[:])
                        logZ = cp.tile([1, NB], F32)
                        em_sc = cp.tile([1, NB], F32)
                        tr_sc = cp.tile([1, NB], F32)
                        sten_s = cp.tile([1, NB], F32)
                        with tc.tile_pool(name="ps_f1", bufs=1, space="PSUM") as ps_f1:
                            sumw = ps_f1.tile([1, NB], F32, tag="f1")
                            nc.tensor.matmul(sumw[:], ones45[:], w[:], start=True, stop=True)
                            nc.scalar.activation(logZ[:], sumw[:], AF.Ln)
                        with tc.tile_pool(name="ps_s1", bufs=1, space="PSUM") as ps_s1:
                            S1 = cp.tile([T, N], F32)
                            nc.vector.tensor_mul(S1[:], emT[:], oh[:])
                            s1p = ps_s1.tile([1, N], F32, tag="fbig")
                            for ck in range(4):
                                nc.tensor.matmul(s1p[:, 512 * ck: 512 * ck + 512], ones45[:],
                                                 S1[:, 512 * ck: 512 * ck + 512], start=True, stop=True)
                            nc.vector.tensor_reduce(
                                em_sc[:], s1p.rearrange("p (t b) -> p b t", b=NB),
                                axis=mybir.AxisListType.X, op=ALU.add)
                        with tc.tile_pool(name="ps_R", bufs=1, space="PSUM") as ps_R:
                            Rp_ = ps_R.tile([T, N], F32, tag="fR")
                            for ck in range(4):
                                nc.tensor.matmul(Rp_[:, 512 * ck: 512 * ck + 512], trans_sb[:],
                                                 oh[:, 512 * ck: 512 * ck + 512], start=True, stop=True)
                            S2 = cp.tile([T, N], F32)
                            nc.vector.tensor_mul(S2[:], Rp_[:], oh2[:])
                        with tc.tile_pool(name="ps_s2", bufs=1, space="PSUM") as ps_s2:
                            s2p = ps_s2.tile([1, N], F32, tag="fbig")
                            for ck in range(4):
                                nc.tensor.matmul(s2p[:, 512 * ck: 512 * ck + 512], ones45[:],
                                                 S2[:, 512 * ck: 512 * ck + 512], start=True, stop=True)
                            nc.vector.tensor_reduce(
                                tr_sc[:], s2p.rearrange("p (t b) -> p b t", b=NB),
                                axis=mybir.AxisListType.X, op=ALU.add)
                        with tc.tile_pool(name="ps_f2", bufs=1, space="PSUM") as ps_f2:
                            stp = cp.tile([T, NB], F32)
                            enp = cp.tile([T, NB], F32)
                            nc.vector.tensor_scalar_mul(stp[:], oh[:, 0:NB], stend[:, 0:1])
                            nc.vector.tensor_scalar_mul(enp[:], oh[:, NB * (S - 1):N], stend[:, 1:2])
                            sten = ps_f2.tile([1, NB], F32, tag="f2")
                            nc.tensor.matmul(sten[:], ones45[:], stp[:], start=True, stop=False)
                            nc.tensor.matmul(sten[:], ones45[:], enp[:], start=False, stop=True)
                            nc.vector.tensor_copy(sten_s[:], sten[:])

                        sc1 = cp.tile([1, NB], F32)
                        sc2 = cp.tile([1, NB], F32)
                        lossa = cp.tile([1, NB], F32)
                        lossb = cp.tile([1, NB], F32)
                        nc.vector.tensor_add(sc1[:], em_sc[:], tr_sc[:])
                        nc.vector.tensor_add(sc2[:], sc1[:], sten_s[:])
                        nc.vector.tensor_tensor(out=lossa[:], in0=logZ[:], in1=sc2[:], op=ALU.subtract)
                        nc.scalar.activation(lossb[:], lossa[:], AF.Copy, bias=(S - 1) * LN45)
                        nc.sync.dma_start(out=d_loss[:], in_=lossb[:])

    nc.finalize()
    return nc


def _perm_gates(w):
    i, f, g, o = np.split(w, 4, axis=0)
    return np.concatenate([i, f, o, g], axis=0)


def _pack_k(wT, kchunks):
    # [K, M] -> [128, kchunks*M] with k-chunk kc at col block kc
    K, M = wT.shape
    assert K == 128 * kchunks
    return np.ascontiguousarray(wT.reshape(kchunks, 128, M).transpose(1, 0, 2).reshape(128, kchunks * M))


def prepare_in_maps(**inputs):
    x = np.asarray(inputs["x"]).astype(np.int32)          # [32, 256]
    tags = np.asarray(inputs["tags"]).astype(np.int32)
    emb = np.asarray(inputs["emb"], dtype=np.float32)
    lin_w = np.asarray(inputs["lin_w"], dtype=np.float32)
    lin_b = np.asarray(inputs["lin_b"], dtype=np.float32)
    start_t = np.asarray(inputs["start_t"], dtype=np.float32)
    end_t = np.asarray(inputs["end_t"], dtype=np.float32)
    trans = np.asarray(inputs["trans"], dtype=np.float32)

    wih = {0: _perm_gates(np.asarray(inputs["w_ih_f"], np.float32)),
           1: _perm_gates(np.asarray(inputs["w_ih_b"], np.float32))}
    whh = {0: _perm_gates(np.asarray(inputs["w_hh_f"], np.float32)),
           1: _perm_gates(np.asarray(inputs["w_hh_b"], np.float32))}
    bb = {0: _perm_gates(np.asarray(inputs["b_f"], np.float32)),
          1: _perm_gates(np.asarray(inputs["b_b"], np.float32))}

    id128 = np.eye(128, dtype=np.float32)
    id8 = np.eye(8, dtype=np.float32)
    u = np.arange(128)
    rev = 8 * (15 - (u // 8)) + (u % 8)
    revM = np.zeros((128, 128), np.float32)
    revM[u, rev[u]] = 1.0
    sel = np.eye(128, dtype=np.float32)   # col 8u+b selects row 8u+b

    in_maps = []
    for core in range(8):
        dirn, grp = core // 4, core % 4
        xs = x[8 * grp: 8 * grp + 8]
        if dirn == 1:
            xs = xs[:, ::-1]
        x_flat = xs.T.reshape(-1)                         # n = 8t+b
        xidx = np.ascontiguousarray(x_flat.reshape(16, 128).T).astype(np.int32)

        tg = tags[8 * grp: 8 * grp + 8]                   # [8, 256]
        oh = np.zeros((T, N), np.float32)
        oh[tg.T.reshape(-1), np.arange(N)] = 1.0
        oh2c = np.zeros((T, N), np.float32)
        oh2c[:, 0:N - NB] = oh[:, NB:N]

        lin_half = lin_w[:, 512 * dirn: 512 * (dirn + 1)]  # [45, 512]
        in_maps.append({
            "emb": emb,
            "xidx": xidx,
            "wihT": _pack_k(wih[dirn].T.copy(), 2),
            "bias1": bb[dirn].reshape(1, G4),
            "whhT": _pack_k(whh[dirn].T.copy(), 4).astype(ml_dtypes.bfloat16),
            "sel": sel.astype(ml_dtypes.bfloat16),
            "id8": id8.astype(ml_dtypes.bfloat16),
            "id128": id128,
            "lin45": _pack_k(np.ascontiguousarray(lin_half.T), 4).astype(ml_dtypes.bfloat16),
            "linb1": (lin_b if dirn == 0 else np.zeros_like(lin_b)).reshape(1, T),
            "keepM": id128 if dirn == 0 else np.zeros_like(id128),
            "swapM": np.zeros_like(revM) if dirn == 0 else revM,
            "trans": trans,
            "stend": np.stack([start_t, end_t], axis=1),
            "oh": oh if dirn == 0 else np.zeros_like(oh),
            "oh2": oh2c if dirn == 0 else np.zeros_like(oh),
        })

    return in_maps


def get_nc():
    if "nc" not in _cached:
        _cached["nc"] = _build()
    return _cached["nc"]


def kernel(**inputs):
    in_maps = prepare_in_maps(**inputs)
    res = run_bass_kernel_spmd(get_nc(), in_maps, core_ids=list(range(8)))
    total = np.float64(0.0)
    for core in range(4):
        total += np.float64(res.results[core]["loss"]).sum()
    return np.float32(total / 32.0)
